# revision 6
# baseline (speedup 1.0000x reference)
"""Trainium2 Bass kernel for prefix-LM CausalSelfAttention.

Problem: B=2, T=2048, C=2048, H=16 heads (hd=128), prefix-LM mask
(bidirectional over first half, causal after), RoPE on q/k.

Sharding over 8 cores: data-parallel on batch (2) x tensor-parallel on
heads (4 heads per core). Each core computes a partial output projection
(its heads' contribution); partials (bf16) are summed on host.

Weight/activation matmul operands are bf16: bf16 stationary operands get
fast-weight-load (83ns vs 173ns f32r). Softmax probabilities stay f32r
(ACT-written bf16 streams ~40% slower as the MOVING matmul operand;
f32r moving with free dim >= 256 runs at 1 row/cycle). PSUM is f32.

Per-core dataflow:
  A. qT/kT = W_{q,k}^T @ x^T   [hd*4, T] transposed layout (head-major).
     DMAs are consolidated (one 3D DMA for all w k-tiles, one per x
     chunk) and split across the Sync and Activation DGE queues.
     PE warm-up matmuls run during the head DMA wait (p-state ramp).
  B. RoPE via pair-swap permutation matmul + DVE combine, interleaved
     with (C) so the PE stays fed while DVE does the rope math
  C. v = x @ Wv   [T, hd*4] natural layout (x tiles as stationary);
     v stays f32r (walrus rejects bf16-stationary x f32r-moving mixes)
  D. attention, query-chunk outer / head inner, software-pipelined:
     S(j+1) issues before PV(j) so the exp latency hides behind PE
     work. Per 512-wide chunk I:
       S'[J] = k_rope[:,J]^T-tile x q_rope[:,I]    (scores transposed)
       P'[J] = exp(S' / sqrt(hd))                  (ACT, PSUM->SBUF f32r)
       diagonal tiles: in-place DVE multiply of the single 128x128
       triangle block (mask is the same triangle for every diag tile)
       y_psum += v[J,h]^T-as-lhsT x P'[J]
       d_psum += ones^T x P'[J]                    (denominator)
     normalize y^T by broadcast fast-reciprocal of d
  E. partial out = yT^T-as-lhsT @ Wp; interleaved into the NEXT query
     chunk's attention stream; rows staged in SBUF as bf16 and DMA'd
     per 128-token row block (halves HBM write traffic).

Fully-masked key tiles are skipped (structural sparsity: 44/64 tiles/head).
"""
import math

import numpy as np

N_HEAD = 16
B = 2
T = 2048
C = 2048
HD = 128
HPC = 4          # heads per core
CL = HPC * HD    # local C = 512
TC = 512         # chunk width (matmul moving free dim / psum bank)
NT = T // TC     # 4 chunks
KT = C // 128    # 16 contraction tiles over C
TT = T // 128    # 16 T tiles
SCALE = 1.0 / math.sqrt(HD)

# Per query-chunk I: list of (J, mask_idx) key tiles to compute.
_JLISTS = {
    0: [(j, None) for j in range(8)],
    1: [(j, None) for j in range(8)],
    2: [(j, None) for j in range(8)] + [(8 + d, d) for d in range(4)],
    3: [(j, None) for j in range(12)] + [(12 + d, d) for d in range(4)],
}

_CACHE = {}


def _build_nc():
    import concourse.tile as tile
    import concourse.mybir as mybir
    from concourse import bacc

    f32 = mybir.dt.float32
    bf16 = mybir.dt.bfloat16

    nc = bacc.Bacc(None, target_bir_lowering=False)

    xT = nc.dram_tensor("xT", [C, T], bf16, kind="ExternalInput")
    wqk = nc.dram_tensor("wqk", [C, 2 * CL], bf16, kind="ExternalInput")
    wv = nc.dram_tensor("wv", [C, CL], bf16, kind="ExternalInput")
    wp = nc.dram_tensor("wp", [CL, C], bf16, kind="ExternalInput")
    cosP = nc.dram_tensor("cosP", [HD, T], bf16, kind="ExternalInput")
    sinP = nc.dram_tensor("sinP", [HD, T], bf16, kind="ExternalInput")
    f32r = mybir.dt.float32r
    rt = nc.dram_tensor("rt", [HD, HD], bf16, kind="ExternalInput")
    tri = nc.dram_tensor("tri", [128, 128], f32r, kind="ExternalInput")
    ones = nc.dram_tensor("ones", [128, 1], f32r, kind="ExternalInput")
    out = nc.dram_tensor("out", [T, C], bf16, kind="ExternalOutput")

    xT3 = xT.rearrange("(kt p) t -> p kt t", p=128)
    wqk3 = wqk.rearrange("(kt p) m -> p kt m", p=128)
    wv3 = wv.rearrange("(kt p) m -> p kt m", p=128)
    wp3 = wp.rearrange("(kt p) m -> p kt m", p=128)

    Exp = mybir.ActivationFunctionType.Exp

    with tile.TileContext(nc) as tc:
        mpool = tc.alloc_tile_pool(name="misc", bufs=1)
        qk_pool = tc.alloc_tile_pool(name="qkrope", bufs=1)
        tpool = tc.alloc_tile_pool(name="trig", bufs=1, side="right")
        xpool = tc.alloc_tile_pool(name="xt_sb", bufs=1, side="right")

        rt_sb = mpool.tile([HD, HD], bf16)
        ones_sb = mpool.tile([128, 1], f32r)
        tri_sb = mpool.tile([128, 128], f32r)
        cos_sb = tpool.tile([HD, T], bf16)
        sin_sb = tpool.tile([HD, T], bf16)
        warm_sb = mpool.tile([1, 1], f32)

        qkT = [qk_pool.tile([128, T], bf16, tag=f"qk{m}", name=f"qk{m}") for m in range(8)]

        # ---- stage A: qT/kT = W_{q,k}^T @ x^T, head-major tiles ----
        # Consolidated DMAs: rt first (tiny, gates PE warm-up), then the
        # critical x chunk-0 + all-w 3D DMAs, then x chunks 1..3. The
        # Activation DGE queue carries everything stage B/C/E needs.
        wpool = tc.alloc_tile_pool(name="wqk_sb", bufs=1)
        ps1 = tc.alloc_tile_pool(name="ps_qk", bufs=8, space="PSUM")
        # Many small DMAs beat one big one (~4x: more outstanding
        # descriptors in the DGE). Critical set for the first chain is
        # all w k-tiles + x chunk-0: w rides the Sync queue, x chunk-0
        # rides the Activation queue in parallel.
        nc.sync.dma_start(out=rt_sb, in_=rt[:, :])
        w_all = wpool.tile([128, KT, 2 * CL], bf16)
        x_all = xpool.tile([128, KT, T], bf16)
        for k in range(KT):
            nc.scalar.dma_start(out=x_all[:, k, 0:TC], in_=xT3[:, k, 0:TC])
            nc.sync.dma_start(out=w_all[:, k], in_=wqk3[:, k])
        for k in range(KT):
            nc.sync.dma_start(out=x_all[:, k, TC:], in_=xT3[:, k, TC:])
        w_t = [w_all[:, k] for k in range(KT)]
        x_t = [x_all[:, k] for k in range(KT)]

        # secondary inputs on the Activation DGE queue (parallel to sync)
        nc.scalar.dma_start(out=cos_sb, in_=cosP[:, :])
        nc.scalar.dma_start(out=sin_sb, in_=sinP[:, :])
        nc.scalar.dma_start(out=ones_sb, in_=ones[:, :])
        nc.scalar.dma_start(out=tri_sb, in_=tri[:, :])

        # warm the ACT exp table during stage A (one-time ~2.7us load)
        nc.scalar.activation(out=warm_sb, in_=rt_sb[0:1, 0:1], func=Exp)
        # PE p-state warm-up: ~4us of dummy matmuls while the w/x DMAs
        # land. rt_sb arrives first (tiny DMA issued before the big ones).
        for wi in range(16):
            ps = ps1.tile([128, TC], f32, tag="ps_qk", name="warm")
            nc.tensor.matmul(ps[:, 0:HD], rt_sb, rt_sb, start=True, stop=True)

        for n in range(NT):
            nsl = slice(n * TC, (n + 1) * TC)
            for m in range(8):
                ps = ps1.tile([128, TC], f32, tag="ps_qk", name="ps_qk")
                for k in range(KT):
                    nc.tensor.matmul(
                        ps, w_t[k][:, m * 128:(m + 1) * 128], x_t[k][:, nsl],
                        start=(k == 0), stop=(k == KT - 1),
                    )
                nc.vector.tensor_copy(out=qkT[m][:, nsl], in_=ps)
        wpool.release()
        ps1.release()

        # ---- stage B+C interleaved: RoPE (PE tiny, DVE heavy) and
        # v = x @ Wv (PE heavy). Emitting v matmuls after each head's rope
        # keeps the PE busy while DVE works through the rope muls.
        v_pool = tc.alloc_tile_pool(name="v_sb", bufs=1)
        wvpool = tc.alloc_tile_pool(name="wv_sb", bufs=1)
        v_t = [v_pool.tile([128, CL], f32r, tag=f"v{mt}", name=f"v{mt}")
               for mt in range(TT)]
        wv_all = wvpool.tile([128, KT, CL], bf16)
        nc.scalar.dma_start(out=wv_all, in_=wv3)
        wv_t = [wv_all[:, k] for k in range(KT)]

        rope = [None] * 8
        rtmp = tc.alloc_tile_pool(name="rope_tmp", bufs=4)
        psr = tc.alloc_tile_pool(name="ps_rot", bufs=4, space="PSUM")
        ps2 = tc.alloc_tile_pool(name="ps_v", bufs=4, space="PSUM")

        def emit_v_pair(pair):
            for half in range(2):
                mt = 2 * pair + half
                tsl = slice(mt * 128, (mt + 1) * 128)
                ps = ps2.tile([128, CL], f32, tag="ps_v", name="ps_v")
                for k in range(KT):
                    nc.tensor.matmul(
                        ps, x_t[k][:, tsl], wv_t[k],
                        start=(k == 0), stop=(k == KT - 1),
                    )
                nc.scalar.copy(out=v_t[mt], in_=ps)

        for idx, m in enumerate((0, 4, 1, 5, 2, 6, 3, 7)):
            tmp = []
            for n in range(NT):
                sl = slice(n * TC, (n + 1) * TC)
                ps = psr.tile([128, TC], f32, tag="ps_rot", name="ps_rot")
                nc.tensor.matmul(ps, rt_sb, qkT[m][:, sl], start=True, stop=True)
                t1 = rtmp.tile([128, TC], bf16, tag="t1", name="t1")
                t2 = rtmp.tile([128, TC], bf16, tag="t2", name="t2")
                nc.vector.tensor_mul(t1, ps, sin_sb[:, sl])
                nc.vector.tensor_mul(t2, qkT[m][:, sl], cos_sb[:, sl])
                tmp.append((t1, t2))
            ro = qk_pool.tile([128, T], bf16, tag=f"qk{m}", name=f"rope{m}")
            for n in range(NT):
                sl = slice(n * TC, (n + 1) * TC)
                nc.vector.tensor_add(ro[:, sl], tmp[n][0], tmp[n][1])
            rope[m] = ro
            emit_v_pair(idx)
        ps2.release()
        psr.release()
        rtmp.release()
        wvpool.release()
        xpool.release()
        tpool.release()

        # ---- stage D attention (query-chunk outer) + stage E interleaved ----
        y_pool = tc.alloc_tile_pool(name="yT_sb", bufs=1)
        yT = [y_pool.tile([128, T], bf16, tag=f"yT{h}", name=f"yT{h}")
              for h in range(HPC)]
        wppool = tc.alloc_tile_pool(name="wp_sb", bufs=1)
        wp_all = wppool.tile([128, HPC, C], bf16)
        nc.scalar.dma_start(out=wp_all, in_=wp3)
        wp_t = [wp_all[:, hk] for hk in range(HPC)]

        pp_pool = tc.alloc_tile_pool(name="pp", bufs=6)
        sm_pool = tc.alloc_tile_pool(name="small", bufs=2)
        ps_s = tc.alloc_tile_pool(name="ps_s", bufs=3, space="PSUM")
        ps_y = tc.alloc_tile_pool(name="ps_y", bufs=2, space="PSUM")
        ps_o = tc.alloc_tile_pool(name="ps_o", bufs=1, space="PSUM")
        ps_d = tc.alloc_tile_pool(name="ps_d", bufs=2, space="PSUM")  # two halves
        opool = tc.alloc_tile_pool(name="ostage", bufs=2)

        ready_E = []     # mt values whose yT inputs are complete
        e_state = [None, 0]  # open [ot_tile, next_n] for current mt

        e_pools = [ps_o]
        e_ctr = [0]

        def emit_e_subgroup():
            # one (mt, n) block: 4 accumulating matmuls + copy to the
            # staged out row; DMA the full row after its 4th block.
            if e_state[0] is None:
                if not ready_E:
                    return
                e_state[0] = (ready_E.pop(0),
                              opool.tile([128, C], bf16, tag="ot", name="ot"))
                e_state[1] = 0
            mt, ot = e_state[0]
            n = e_state[1]
            msl = slice(mt * 128, (mt + 1) * 128)
            pool = e_pools[e_ctr[0] % len(e_pools)]
            e_ctr[0] += 1
            ps = pool.tile([128, TC], f32, tag="o", name="o_ps")
            for hk in range(HPC):
                nc.tensor.matmul(
                    ps, yT[hk][:, msl], wp_t[hk][:, n * TC:(n + 1) * TC],
                    start=(hk == 0), stop=(hk == HPC - 1),
                )
            nc.vector.tensor_copy(out=ot[:, n * TC:(n + 1) * TC], in_=ps)
            if mt >= TT - 4:
                nc.sync.dma_start(out=out[msl, n * TC:(n + 1) * TC],
                                  in_=ot[:, n * TC:(n + 1) * TC])
            e_state[1] += 1
            if e_state[1] == NT:
                if mt < TT - 4:
                    # interior rows ride the gpsimd software DGE so the
                    # final blocks' sync-queue DMAs don't queue behind them
                    nc.gpsimd.dma_start(out=out[msl, :], in_=ot)
                e_state[0] = None

        for I in range(NT):
            isl = slice(I * TC, (I + 1) * TC)
            jl = _JLISTS[I]
            for h in range(HPC):
                q_h = rope[h]
                k_h = rope[4 + h]
                y_ps = ps_y.tile([128, TC], f32, tag="y", name="y_ps")
                d_ps = ps_d.tile([1, TC], f32, tag="d", name="d_ps")
                pps = []

                def emit_S(jidx):
                    J, dmi = jl[jidx]
                    off = 0 if dmi is None else dmi * 128
                    osl = slice(off, TC)
                    s_ps = ps_s.tile([128, TC], f32, tag="s", name="s_ps")
                    nc.tensor.matmul(
                        s_ps[:, osl], k_h[:, J * 128:(J + 1) * 128],
                        q_h[:, I * TC + off:(I + 1) * TC],
                        start=True, stop=True,
                    )
                    pp = pp_pool.tile([128, TC], f32r, tag="pp", name="pp")
                    nc.scalar.activation(out=pp[:, osl], in_=s_ps[:, osl],
                                         func=Exp, scale=SCALE)
                    if dmi is not None:
                        # in-place masked multiply on the single 128x128
                        # triangle block at the streamed range's head
                        dsl = slice(off, off + 128)
                        nc.vector.tensor_mul(pp[:, dsl], pp[:, dsl], tri_sb)
                    pps.append((pp, osl, J, dmi))

                emit_S(0)
                for jidx in range(len(jl)):
                    if jidx + 1 < len(jl):
                        emit_S(jidx + 1)
                    pp, osl, J, dmi = pps[jidx]
                    if dmi is not None:
                        # extra PE fill while the DVE mask-mul lands
                        emit_e_subgroup()
                    first = jidx == 0
                    last = jidx == len(jl) - 1
                    nc.tensor.matmul(
                        y_ps[:, osl], v_t[J][:, h * 128:(h + 1) * 128],
                        pp[:, osl], start=first, stop=last,
                    )
                    nc.tensor.matmul(d_ps[:, osl], ones_sb, pp[:, osl],
                                     start=first, stop=last)
                    if jidx % 2 == 1 or len(ready_E) > 8:
                        emit_e_subgroup()
                recip = sm_pool.tile([1, TC], f32, tag="recip", name="recip")
                nc.vector.reciprocal_approx_fast(out=recip, in_=d_ps)
                recipB = sm_pool.tile([128, TC], f32, tag="recipB", name="recipB")
                nc.gpsimd.partition_broadcast(recipB, recip)
                nc.vector.tensor_mul(yT[h][:, isl], y_ps, recipB)
            # all heads' columns for chunk I are now complete
            ready_E.extend(range(4 * I, 4 * I + 4))
        ps_d.release()
        ps_o2 = tc.alloc_tile_pool(name="ps_o2", bufs=1, space="PSUM")
        e_pools.append(ps_o2)
        while ready_E or e_state[0] is not None:
            emit_e_subgroup()

        for p in (opool, sm_pool, pp_pool, wppool, y_pool, v_pool,
                  qk_pool, mpool, ps_o2, ps_o, ps_y, ps_s):
            p.release()
    nc.compile()
    return nc


def _host_prep(x, w_qkv, w_proj, freqs_cis):
    """Build per-core input maps (slicing + layout prep only)."""
    try:
        import ml_dtypes
        bf = ml_dtypes.bfloat16
    except ImportError:
        import jax.numpy as jnp
        bf = jnp.bfloat16
    x = np.asarray(x, dtype=np.float32)
    w_qkv = np.asarray(w_qkv, dtype=np.float32)
    w_proj = np.asarray(w_proj, dtype=np.float32)
    fc = np.asarray(freqs_cis, dtype=np.float32)

    xTb = [np.ascontiguousarray(x[b].T).astype(bf) for b in range(B)]

    cos = fc[:, :, 0].T  # [64, T]
    sin = fc[:, :, 1].T
    cosP = np.repeat(cos, 2, axis=0).astype(bf)  # [128, T]
    sinP = np.repeat(sin, 2, axis=0).astype(bf)

    rt = np.zeros((HD, HD), dtype=np.float32)
    for d in range(HD // 2):
        rt[2 * d, 2 * d + 1] = 1.0
        rt[2 * d + 1, 2 * d] = -1.0
    rt = rt.astype(bf)

    ii = np.arange(128)[None, :]
    jj = np.arange(128)[:, None]
    tri = (ii >= jj).astype(np.float32)

    ones = np.ones((128, 1), dtype=np.float32)

    in_maps = []
    for core in range(8):
        b = core // 4
        g = core % 4
        qc = w_qkv[:, 512 * g: 512 * (g + 1)]
        kc = w_qkv[:, 2048 + 512 * g: 2048 + 512 * (g + 1)]
        vc = np.ascontiguousarray(w_qkv[:, 4096 + 512 * g: 4096 + 512 * (g + 1)]).astype(bf)
        wqk_c = np.concatenate([qc, kc], axis=1).astype(bf)
        wp_c = np.ascontiguousarray(w_proj[512 * g: 512 * (g + 1), :]).astype(bf)
        in_maps.append({
            "xT": xTb[b],
            "wqk": wqk_c,
            "wv": vc,
            "wp": wp_c,
            "cosP": cosP,
            "sinP": sinP,
            "rt": rt,
            "tri": tri,
            "ones": ones,
        })
    return in_maps


def _get_nc():
    if "nc" not in _CACHE:
        _CACHE["nc"] = _build_nc()
    return _CACHE["nc"]


def kernel(x, w_qkv, w_proj, freqs_cis, attn_mask, _trace=False):
    from concourse.bass_utils import run_bass_kernel_spmd

    in_maps = _host_prep(x, w_qkv, w_proj, freqs_cis)
    nc = _get_nc()
    res = run_bass_kernel_spmd(
        nc, in_maps, core_ids=list(range(8)), trace=_trace,
    )
    outs = [np.asarray(r["out"]).astype(np.float64) for r in res.results]
    full = np.stack([
        outs[0] + outs[1] + outs[2] + outs[3],
        outs[4] + outs[5] + outs[6] + outs[7],
    ]).astype(np.float32)
    if _trace:
        kernel._last_results = res
    return full


# revision 7
# speedup vs baseline: 1.1609x; 1.1609x over previous
"""Trainium2 Bass kernel for prefix-LM CausalSelfAttention.

Problem: B=2, T=2048, C=2048, H=16 heads (hd=128), prefix-LM mask
(bidirectional over first half, causal after), RoPE on q/k.

Sharding over 8 cores: data-parallel on batch (2) x tensor-parallel on
heads (4 heads per core). Each core computes a partial output projection
(its heads' contribution); partials (bf16) are summed on host.

Weight/activation matmul operands are bf16: bf16 stationary operands get
fast-weight-load (83ns vs 173ns f32r). Softmax probabilities stay f32r
(ACT-written bf16 streams ~40% slower as the MOVING matmul operand;
f32r moving with free dim >= 256 runs at 1 row/cycle). PSUM is f32.

Per-core dataflow:
  A. qT/kT = W_{q,k}^T @ x^T   [hd*4, T] transposed layout (head-major).
     DMAs are consolidated (one 3D DMA for all w k-tiles, one per x
     chunk) and split across the Sync and Activation DGE queues.
     PE warm-up matmuls run during the head DMA wait (p-state ramp).
  B. RoPE via pair-swap permutation matmul + DVE combine, interleaved
     with (C) so the PE stays fed while DVE does the rope math
  C. v = x @ Wv   [T, hd*4] natural layout (x tiles as stationary);
     v stays f32r (walrus rejects bf16-stationary x f32r-moving mixes)
  D. attention, query-chunk outer / head inner, software-pipelined:
     S(j+1) issues before PV(j) so the exp latency hides behind PE
     work. Per 512-wide chunk I:
       S'[J] = k_rope[:,J]^T-tile x q_rope[:,I]    (scores transposed)
       P'[J] = exp(S' / sqrt(hd))                  (ACT, PSUM->SBUF f32r)
       diagonal tiles: in-place DVE multiply of the single 128x128
       triangle block (mask is the same triangle for every diag tile)
       y_psum += v[J,h]^T-as-lhsT x P'[J]
       d_psum += ones^T x P'[J]                    (denominator)
     normalize y^T by broadcast fast-reciprocal of d
  E. partial out = yT^T-as-lhsT @ Wp; interleaved into the NEXT query
     chunk's attention stream; rows staged in SBUF as bf16 and DMA'd
     per 128-token row block (halves HBM write traffic).

Fully-masked key tiles are skipped (structural sparsity: 44/64 tiles/head).
"""
import math

import numpy as np

N_HEAD = 16
B = 2
T = 2048
C = 2048
HD = 128
HPC = 4          # heads per core
CL = HPC * HD    # local C = 512
TC = 512         # chunk width (matmul moving free dim / psum bank)
NT = T // TC     # 4 chunks
KT = C // 128    # 16 contraction tiles over C
TT = T // 128    # 16 T tiles
SCALE = 1.0 / math.sqrt(HD)

# Per query-chunk I: list of (J, mask_idx) key tiles to compute.
_JLISTS = {
    0: [(j, None) for j in range(8)],
    1: [(j, None) for j in range(8)],
    2: [(j, None) for j in range(8)] + [(8 + d, d) for d in range(4)],
    3: [(j, None) for j in range(12)] + [(12 + d, d) for d in range(4)],
}

_CACHE = {}


def _build_nc():
    import concourse.tile as tile
    import concourse.mybir as mybir
    from concourse import bacc

    f32 = mybir.dt.float32
    bf16 = mybir.dt.bfloat16

    nc = bacc.Bacc(None, target_bir_lowering=False)

    xT = nc.dram_tensor("xT", [C, T], bf16, kind="ExternalInput")
    wqk = nc.dram_tensor("wqk", [C, 2 * CL], bf16, kind="ExternalInput")
    wv = nc.dram_tensor("wv", [C, CL], bf16, kind="ExternalInput")
    wp = nc.dram_tensor("wp", [CL, C], bf16, kind="ExternalInput")
    cosP = nc.dram_tensor("cosP", [HD, T], bf16, kind="ExternalInput")
    sinP = nc.dram_tensor("sinP", [HD, T], bf16, kind="ExternalInput")
    f32r = mybir.dt.float32r
    rt = nc.dram_tensor("rt", [HD, HD], bf16, kind="ExternalInput")
    tri = nc.dram_tensor("tri", [128, 128], f32r, kind="ExternalInput")
    ones = nc.dram_tensor("ones", [128, 1], f32r, kind="ExternalInput")
    out = nc.dram_tensor("out", [T, C], bf16, kind="ExternalOutput")

    xT3 = xT.rearrange("(kt p) t -> p kt t", p=128)
    wqk3 = wqk.rearrange("(kt p) m -> p kt m", p=128)
    wv3 = wv.rearrange("(kt p) m -> p kt m", p=128)
    wp3 = wp.rearrange("(kt p) m -> p kt m", p=128)

    Exp = mybir.ActivationFunctionType.Exp

    with tile.TileContext(nc) as tc:
        mpool = tc.alloc_tile_pool(name="misc", bufs=1)
        qk_pool = tc.alloc_tile_pool(name="qkrope", bufs=1)
        tpool = tc.alloc_tile_pool(name="trig", bufs=1, side="right")
        xpool = tc.alloc_tile_pool(name="xt_sb", bufs=1, side="right")

        rt_sb = mpool.tile([HD, HD], bf16)
        ones_sb = mpool.tile([128, 1], f32r)
        tri_sb = mpool.tile([128, 128], f32r)
        cos_sb = tpool.tile([HD, T], bf16)
        sin_sb = tpool.tile([HD, T], bf16)
        warm_sb = mpool.tile([1, 1], f32)

        qkT = [qk_pool.tile([128, T], bf16, tag=f"qk{m}", name=f"qk{m}") for m in range(8)]

        # ---- stage A: qT/kT = W_{q,k}^T @ x^T, head-major tiles ----
        # Consolidated DMAs: rt first (tiny, gates PE warm-up), then the
        # critical x chunk-0 + all-w 3D DMAs, then x chunks 1..3. The
        # Activation DGE queue carries everything stage B/C/E needs.
        wpool = tc.alloc_tile_pool(name="wqk_sb", bufs=1)
        ps1 = tc.alloc_tile_pool(name="ps_qk", bufs=8, space="PSUM")
        # Many small DMAs beat one big one (~4x: more outstanding
        # descriptors in the DGE). Critical set for the first chain is
        # all w k-tiles + x chunk-0: w rides the Sync queue, x chunk-0
        # rides the Activation queue in parallel.
        nc.sync.dma_start(out=rt_sb, in_=rt[:, :])
        w_all = wpool.tile([128, KT, 2 * CL], bf16)
        x_all = xpool.tile([128, KT, T], bf16)
        for k in range(KT):
            nc.scalar.dma_start(out=x_all[:, k, 0:TC], in_=xT3[:, k, 0:TC])
            nc.sync.dma_start(out=w_all[:, k], in_=wqk3[:, k])
        for k in range(KT):
            nc.sync.dma_start(out=x_all[:, k, TC:], in_=xT3[:, k, TC:])
        w_t = [w_all[:, k] for k in range(KT)]
        x_t = [x_all[:, k] for k in range(KT)]

        # secondary inputs on the Activation DGE queue (parallel to sync)
        nc.scalar.dma_start(out=cos_sb, in_=cosP[:, :])
        nc.scalar.dma_start(out=sin_sb, in_=sinP[:, :])
        nc.scalar.dma_start(out=ones_sb, in_=ones[:, :])
        nc.scalar.dma_start(out=tri_sb, in_=tri[:, :])

        # warm the ACT exp table during stage A (one-time ~2.7us load)
        nc.scalar.activation(out=warm_sb, in_=rt_sb[0:1, 0:1], func=Exp)
        # PE p-state warm-up: ~4us of dummy matmuls while the w/x DMAs
        # land. rt_sb arrives first (tiny DMA issued before the big ones).
        for wi in range(16):
            ps = ps1.tile([128, TC], f32, tag="ps_qk", name="warm")
            nc.tensor.matmul(ps[:, 0:HD], rt_sb, rt_sb, start=True, stop=True)

        for n in range(NT):
            nsl = slice(n * TC, (n + 1) * TC)
            for m in range(8):
                ps = ps1.tile([128, TC], f32, tag="ps_qk", name="ps_qk")
                for k in range(KT):
                    nc.tensor.matmul(
                        ps, w_t[k][:, m * 128:(m + 1) * 128], x_t[k][:, nsl],
                        start=(k == 0), stop=(k == KT - 1),
                    )
                nc.vector.tensor_copy(out=qkT[m][:, nsl], in_=ps)
        wpool.release()
        ps1.release()

        # ---- stage B+C interleaved: RoPE (PE tiny, DVE heavy) and
        # v = x @ Wv (PE heavy). Emitting v matmuls after each head's rope
        # keeps the PE busy while DVE works through the rope muls.
        v_pool = tc.alloc_tile_pool(name="v_sb", bufs=1)
        wvpool = tc.alloc_tile_pool(name="wv_sb", bufs=1)
        v_t = [v_pool.tile([128, CL], f32r, tag=f"v{mt}", name=f"v{mt}")
               for mt in range(TT)]
        wv_all = wvpool.tile([128, KT, CL], bf16)
        nc.scalar.dma_start(out=wv_all, in_=wv3)
        wv_t = [wv_all[:, k] for k in range(KT)]

        rope = [None] * 8
        rtmp = tc.alloc_tile_pool(name="rope_tmp", bufs=4)
        psr = tc.alloc_tile_pool(name="ps_rot", bufs=4, space="PSUM")
        ps2 = tc.alloc_tile_pool(name="ps_v", bufs=4, space="PSUM")

        def emit_v_pair(pair):
            for half in range(2):
                mt = 2 * pair + half
                tsl = slice(mt * 128, (mt + 1) * 128)
                ps = ps2.tile([128, CL], f32, tag="ps_v", name="ps_v")
                for k in range(KT):
                    nc.tensor.matmul(
                        ps, x_t[k][:, tsl], wv_t[k],
                        start=(k == 0), stop=(k == KT - 1),
                    )
                nc.scalar.copy(out=v_t[mt], in_=ps)

        for idx, m in enumerate((0, 4, 1, 5, 2, 6, 3, 7)):
            tmp = []
            for n in range(NT):
                sl = slice(n * TC, (n + 1) * TC)
                ps = psr.tile([128, TC], f32, tag="ps_rot", name="ps_rot")
                nc.tensor.matmul(ps, rt_sb, qkT[m][:, sl], start=True, stop=True)
                t1 = rtmp.tile([128, TC], bf16, tag="t1", name="t1")
                t2 = rtmp.tile([128, TC], bf16, tag="t2", name="t2")
                nc.vector.tensor_mul(t1, ps, sin_sb[:, sl])
                nc.vector.tensor_mul(t2, qkT[m][:, sl], cos_sb[:, sl])
                tmp.append((t1, t2))
            ro = qk_pool.tile([128, T], bf16, tag=f"qk{m}", name=f"rope{m}")
            for n in range(NT):
                sl = slice(n * TC, (n + 1) * TC)
                nc.vector.tensor_add(ro[:, sl], tmp[n][0], tmp[n][1])
            rope[m] = ro
            emit_v_pair(idx)
        ps2.release()
        psr.release()
        rtmp.release()
        wvpool.release()
        xpool.release()
        tpool.release()

        # ---- stage D attention (query-chunk outer) + stage E interleaved ----
        y_pool = tc.alloc_tile_pool(name="yT_sb", bufs=1)
        yT = [y_pool.tile([128, T], bf16, tag=f"yT{h}", name=f"yT{h}")
              for h in range(HPC)]
        wppool = tc.alloc_tile_pool(name="wp_sb", bufs=1)
        wp_all = wppool.tile([128, HPC, C], bf16)
        nc.scalar.dma_start(out=wp_all, in_=wp3)
        wp_t = [wp_all[:, hk] for hk in range(HPC)]

        pp_pool = tc.alloc_tile_pool(name="pp", bufs=6)
        sm_pool = tc.alloc_tile_pool(name="small", bufs=2)
        ps_s = tc.alloc_tile_pool(name="ps_s", bufs=3, space="PSUM")
        ps_y = tc.alloc_tile_pool(name="ps_y", bufs=2, space="PSUM")
        ps_o = tc.alloc_tile_pool(name="ps_o", bufs=1, space="PSUM")
        ps_d = tc.alloc_tile_pool(name="ps_d", bufs=2, space="PSUM")  # two halves
        opool = tc.alloc_tile_pool(name="ostage", bufs=2)

        ready_E = []     # mt values whose yT inputs are complete
        e_state = [None, 0]  # open [ot_tile, next_n] for current mt

        e_pools = [ps_o]
        e_ctr = [0]

        def emit_e_subgroup():
            # one (mt, n) block: 4 accumulating matmuls + copy to the
            # staged out row; DMA the full row after its 4th block.
            if e_state[0] is None:
                if not ready_E:
                    return
                e_state[0] = (ready_E.pop(0),
                              opool.tile([128, C], bf16, tag="ot", name="ot"))
                e_state[1] = 0
            mt, ot = e_state[0]
            n = e_state[1]
            msl = slice(mt * 128, (mt + 1) * 128)
            pool = e_pools[e_ctr[0] % len(e_pools)]
            e_ctr[0] += 1
            ps = pool.tile([128, TC], f32, tag="o", name="o_ps")
            for hk in range(HPC):
                nc.tensor.matmul(
                    ps, yT[hk][:, msl], wp_t[hk][:, n * TC:(n + 1) * TC],
                    start=(hk == 0), stop=(hk == HPC - 1),
                )
            nc.vector.tensor_copy(out=ot[:, n * TC:(n + 1) * TC], in_=ps)
            if mt >= TT - 4:
                nc.sync.dma_start(out=out[msl, n * TC:(n + 1) * TC],
                                  in_=ot[:, n * TC:(n + 1) * TC])
            e_state[1] += 1
            if e_state[1] == NT:
                if mt < TT - 4:
                    # interior rows ride the Activation DGE queue so the
                    # final blocks' sync-queue DMAs don't queue behind them
                    nc.scalar.dma_start(out=out[msl, :], in_=ot)
                e_state[0] = None

        for I in range(NT):
            isl = slice(I * TC, (I + 1) * TC)
            jl = _JLISTS[I]
            for h in range(HPC):
                q_h = rope[h]
                k_h = rope[4 + h]
                y_ps = ps_y.tile([128, TC], f32, tag="y", name="y_ps")
                d_ps = ps_d.tile([1, TC], f32, tag="d", name="d_ps")
                pps = []

                def emit_S(jidx):
                    J, dmi = jl[jidx]
                    off = 0 if dmi is None else dmi * 128
                    osl = slice(off, TC)
                    s_ps = ps_s.tile([128, TC], f32, tag="s", name="s_ps")
                    nc.tensor.matmul(
                        s_ps[:, osl], k_h[:, J * 128:(J + 1) * 128],
                        q_h[:, I * TC + off:(I + 1) * TC],
                        start=True, stop=True,
                    )
                    pp = pp_pool.tile([128, TC], f32r, tag="pp", name="pp")
                    nc.scalar.activation(out=pp[:, osl], in_=s_ps[:, osl],
                                         func=Exp, scale=SCALE)
                    if dmi is not None:
                        # in-place masked multiply on the single 128x128
                        # triangle block at the streamed range's head
                        dsl = slice(off, off + 128)
                        nc.vector.tensor_mul(pp[:, dsl], pp[:, dsl], tri_sb)
                    pps.append((pp, osl, J, dmi))

                emit_S(0)
                for jidx in range(len(jl)):
                    if jidx + 1 < len(jl):
                        emit_S(jidx + 1)
                    pp, osl, J, dmi = pps[jidx]
                    if dmi is not None:
                        # extra PE fill while the DVE mask-mul lands
                        emit_e_subgroup()
                    first = jidx == 0
                    last = jidx == len(jl) - 1
                    nc.tensor.matmul(
                        y_ps[:, osl], v_t[J][:, h * 128:(h + 1) * 128],
                        pp[:, osl], start=first, stop=last,
                    )
                    nc.tensor.matmul(d_ps[:, osl], ones_sb, pp[:, osl],
                                     start=first, stop=last)
                    if jidx % 2 == 1 or len(ready_E) > 8:
                        emit_e_subgroup()
                recip = sm_pool.tile([1, TC], f32, tag="recip", name="recip")
                nc.vector.reciprocal_approx_fast(out=recip, in_=d_ps)
                recipB = sm_pool.tile([128, TC], f32, tag="recipB", name="recipB")
                nc.gpsimd.partition_broadcast(recipB, recip)
                nc.vector.tensor_mul(yT[h][:, isl], y_ps, recipB)
            # all heads' columns for chunk I are now complete
            ready_E.extend(range(4 * I, 4 * I + 4))
        ps_d.release()
        ps_o2 = tc.alloc_tile_pool(name="ps_o2", bufs=1, space="PSUM")
        e_pools.append(ps_o2)
        while ready_E or e_state[0] is not None:
            emit_e_subgroup()

        for p in (opool, sm_pool, pp_pool, wppool, y_pool, v_pool,
                  qk_pool, mpool, ps_o2, ps_o, ps_y, ps_s):
            p.release()
    nc.compile()
    return nc


def _host_prep(x, w_qkv, w_proj, freqs_cis):
    """Build per-core input maps (slicing + layout prep only)."""
    try:
        import ml_dtypes
        bf = ml_dtypes.bfloat16
    except ImportError:
        import jax.numpy as jnp
        bf = jnp.bfloat16
    x = np.asarray(x, dtype=np.float32)
    w_qkv = np.asarray(w_qkv, dtype=np.float32)
    w_proj = np.asarray(w_proj, dtype=np.float32)
    fc = np.asarray(freqs_cis, dtype=np.float32)

    xTb = [np.ascontiguousarray(x[b].T).astype(bf) for b in range(B)]

    cos = fc[:, :, 0].T  # [64, T]
    sin = fc[:, :, 1].T
    cosP = np.repeat(cos, 2, axis=0).astype(bf)  # [128, T]
    sinP = np.repeat(sin, 2, axis=0).astype(bf)

    rt = np.zeros((HD, HD), dtype=np.float32)
    for d in range(HD // 2):
        rt[2 * d, 2 * d + 1] = 1.0
        rt[2 * d + 1, 2 * d] = -1.0
    rt = rt.astype(bf)

    ii = np.arange(128)[None, :]
    jj = np.arange(128)[:, None]
    tri = (ii >= jj).astype(np.float32)

    ones = np.ones((128, 1), dtype=np.float32)

    in_maps = []
    for core in range(8):
        b = core // 4
        g = core % 4
        qc = w_qkv[:, 512 * g: 512 * (g + 1)]
        kc = w_qkv[:, 2048 + 512 * g: 2048 + 512 * (g + 1)]
        vc = np.ascontiguousarray(w_qkv[:, 4096 + 512 * g: 4096 + 512 * (g + 1)]).astype(bf)
        wqk_c = np.concatenate([qc, kc], axis=1).astype(bf)
        wp_c = np.ascontiguousarray(w_proj[512 * g: 512 * (g + 1), :]).astype(bf)
        in_maps.append({
            "xT": xTb[b],
            "wqk": wqk_c,
            "wv": vc,
            "wp": wp_c,
            "cosP": cosP,
            "sinP": sinP,
            "rt": rt,
            "tri": tri,
            "ones": ones,
        })
    return in_maps


def _get_nc():
    if "nc" not in _CACHE:
        _CACHE["nc"] = _build_nc()
    return _CACHE["nc"]


def kernel(x, w_qkv, w_proj, freqs_cis, attn_mask, _trace=False):
    from concourse.bass_utils import run_bass_kernel_spmd

    in_maps = _host_prep(x, w_qkv, w_proj, freqs_cis)
    nc = _get_nc()
    res = run_bass_kernel_spmd(
        nc, in_maps, core_ids=list(range(8)), trace=_trace,
    )
    outs = [np.asarray(r["out"]).astype(np.float64) for r in res.results]
    full = np.stack([
        outs[0] + outs[1] + outs[2] + outs[3],
        outs[4] + outs[5] + outs[6] + outs[7],
    ]).astype(np.float32)
    if _trace:
        kernel._last_results = res
    return full


# revision 10
# speedup vs baseline: 1.1722x; 1.0097x over previous
"""Trainium2 Bass kernel for prefix-LM CausalSelfAttention.

Problem: B=2, T=2048, C=2048, H=16 heads (hd=128), prefix-LM mask
(bidirectional over first half, causal after), RoPE on q/k.

Sharding over 8 cores: data-parallel on batch (2) x tensor-parallel on
heads (4 heads per core). Each core computes a partial output projection
(its heads' contribution); partials (bf16) are summed on host.

Weight/activation matmul operands are bf16: bf16 stationary operands get
fast-weight-load (83ns vs 173ns f32r). Softmax probabilities stay f32r
(ACT-written bf16 streams ~40% slower as the MOVING matmul operand;
f32r moving with free dim >= 256 runs at 1 row/cycle). PSUM is f32.

Per-core dataflow:
  A. qT/kT = W_{q,k}^T @ x^T   [hd*4, T] transposed layout (head-major).
     DMAs are consolidated (one 3D DMA for all w k-tiles, one per x
     chunk) and split across the Sync and Activation DGE queues.
     PE warm-up matmuls run during the head DMA wait (p-state ramp).
  B. RoPE via pair-swap permutation matmul + DVE combine, interleaved
     with (C) so the PE stays fed while DVE does the rope math
  C. v = x @ Wv   [T, hd*4] natural layout (x tiles as stationary);
     v stays f32r (walrus rejects bf16-stationary x f32r-moving mixes)
  D. attention, query-chunk outer / head inner, software-pipelined:
     S(j+1) issues before PV(j) so the exp latency hides behind PE
     work. Per 512-wide chunk I:
       S'[J] = k_rope[:,J]^T-tile x q_rope[:,I]    (scores transposed)
       P'[J] = exp(S' / sqrt(hd))                  (ACT, PSUM->SBUF f32r)
       diagonal tiles: in-place DVE multiply of the single 128x128
       triangle block (mask is the same triangle for every diag tile)
       y_psum += v[J,h]^T-as-lhsT x P'[J]
       d_psum += ones^T x P'[J]                    (denominator)
     normalize y^T by broadcast fast-reciprocal of d
  E. partial out = yT^T-as-lhsT @ Wp; interleaved into the NEXT query
     chunk's attention stream; rows staged in SBUF as bf16 and DMA'd
     per 128-token row block (halves HBM write traffic).

Fully-masked key tiles are skipped (structural sparsity: 44/64 tiles/head).
"""
import math

import numpy as np

N_HEAD = 16
B = 2
T = 2048
C = 2048
HD = 128
HPC = 4          # heads per core
CL = HPC * HD    # local C = 512
TC = 512         # chunk width (matmul moving free dim / psum bank)
NT = T // TC     # 4 chunks
KT = C // 128    # 16 contraction tiles over C
TT = T // 128    # 16 T tiles
SCALE = 1.0 / math.sqrt(HD)

# Per query-chunk I: list of (J, mask_idx) key tiles to compute.
_JLISTS = {
    0: [(j, None) for j in range(8)],
    1: [(j, None) for j in range(8)],
    2: [(j, None) for j in range(8)] + [(8 + d, d) for d in range(4)],
    3: [(j, None) for j in range(12)] + [(12 + d, d) for d in range(4)],
}

_CACHE = {}


def _build_nc():
    import concourse.tile as tile
    import concourse.mybir as mybir
    from concourse import bacc

    f32 = mybir.dt.float32
    bf16 = mybir.dt.bfloat16

    nc = bacc.Bacc(None, target_bir_lowering=False)

    xT = nc.dram_tensor("xT", [C, T], bf16, kind="ExternalInput")
    wqk = nc.dram_tensor("wqk", [C, 2 * CL], bf16, kind="ExternalInput")
    wv = nc.dram_tensor("wv", [C, CL], bf16, kind="ExternalInput")
    wp = nc.dram_tensor("wp", [CL, C], bf16, kind="ExternalInput")
    cosP = nc.dram_tensor("cosP", [HD, T], bf16, kind="ExternalInput")
    sinP = nc.dram_tensor("sinP", [HD, T], bf16, kind="ExternalInput")
    f32r = mybir.dt.float32r
    rt = nc.dram_tensor("rt", [HD, HD], bf16, kind="ExternalInput")
    tri = nc.dram_tensor("tri", [128, 128], f32r, kind="ExternalInput")
    ones = nc.dram_tensor("ones", [128, 1], f32r, kind="ExternalInput")
    out = nc.dram_tensor("out", [T, C], bf16, kind="ExternalOutput")

    xT3 = xT.rearrange("(kt p) t -> p kt t", p=128)
    wqk3 = wqk.rearrange("(kt p) m -> p kt m", p=128)
    wv3 = wv.rearrange("(kt p) m -> p kt m", p=128)
    wp3 = wp.rearrange("(kt p) m -> p kt m", p=128)

    Exp = mybir.ActivationFunctionType.Exp

    with tile.TileContext(nc) as tc:
        mpool = tc.alloc_tile_pool(name="misc", bufs=1)
        qk_pool = tc.alloc_tile_pool(name="qkrope", bufs=1)
        tpool = tc.alloc_tile_pool(name="trig", bufs=1, side="right")
        xpool = tc.alloc_tile_pool(name="xt_sb", bufs=1, side="right")

        rt_sb = mpool.tile([HD, HD], bf16)
        ones_sb = mpool.tile([128, 1], f32r)
        tri_sb = mpool.tile([128, 128], f32r)
        cos_sb = tpool.tile([HD, T], bf16)
        sin_sb = tpool.tile([HD, T], bf16)
        warm_sb = mpool.tile([1, 1], f32)

        qkT = [qk_pool.tile([128, T], bf16, tag=f"qk{m}", name=f"qk{m}") for m in range(8)]

        # ---- stage A: qT/kT = W_{q,k}^T @ x^T, head-major tiles ----
        # Consolidated DMAs: rt first (tiny, gates PE warm-up), then the
        # critical x chunk-0 + all-w 3D DMAs, then x chunks 1..3. The
        # Activation DGE queue carries everything stage B/C/E needs.
        wpool = tc.alloc_tile_pool(name="wqk_sb", bufs=1)
        ps1 = tc.alloc_tile_pool(name="ps_qk", bufs=8, space="PSUM")
        # Many small DMAs beat one big one (~4x: more outstanding
        # descriptors in the DGE). Critical set for the first chain is
        # all w k-tiles + x chunk-0: w rides the Sync queue, x chunk-0
        # rides the Activation queue in parallel.
        nc.sync.dma_start(out=rt_sb, in_=rt[:, :])
        w_all = wpool.tile([128, KT, 2 * CL], bf16)
        x_all = xpool.tile([128, KT, T], bf16)
        for k in range(KT):
            nc.scalar.dma_start(out=x_all[:, k, 0:TC], in_=xT3[:, k, 0:TC])
            nc.sync.dma_start(out=w_all[:, k], in_=wqk3[:, k])
        for k in range(KT):
            nc.sync.dma_start(out=x_all[:, k, TC:], in_=xT3[:, k, TC:])
        w_t = [w_all[:, k] for k in range(KT)]
        x_t = [x_all[:, k] for k in range(KT)]

        # secondary inputs on the Activation DGE queue (parallel to sync)
        nc.scalar.dma_start(out=cos_sb, in_=cosP[:, :])
        nc.scalar.dma_start(out=sin_sb, in_=sinP[:, :])
        nc.scalar.dma_start(out=ones_sb, in_=ones[:, :])
        nc.scalar.dma_start(out=tri_sb, in_=tri[:, :])

        # warm the ACT exp table during stage A (one-time ~2.7us load)
        nc.scalar.activation(out=warm_sb, in_=rt_sb[0:1, 0:1], func=Exp)
        # PE p-state warm-up: ~4us of dummy matmuls while the w/x DMAs
        # land. rt_sb arrives first (tiny DMA issued before the big ones).
        for wi in range(16):
            ps = ps1.tile([128, TC], f32, tag="ps_qk", name="warm")
            nc.tensor.matmul(ps[:, 0:HD], rt_sb, rt_sb, start=True, stop=True)

        for n in range(NT):
            nsl = slice(n * TC, (n + 1) * TC)
            for m in range(8):
                ps = ps1.tile([128, TC], f32, tag="ps_qk", name="ps_qk")
                for k in range(KT):
                    nc.tensor.matmul(
                        ps, w_t[k][:, m * 128:(m + 1) * 128], x_t[k][:, nsl],
                        start=(k == 0), stop=(k == KT - 1),
                    )
                nc.vector.tensor_copy(out=qkT[m][:, nsl], in_=ps)
        wpool.release()
        ps1.release()

        # ---- stage B+C interleaved: RoPE (PE tiny, DVE heavy) and
        # v = x @ Wv (PE heavy). Emitting v matmuls after each head's rope
        # keeps the PE busy while DVE works through the rope muls.
        v_pool = tc.alloc_tile_pool(name="v_sb", bufs=1)
        wvpool = tc.alloc_tile_pool(name="wv_sb", bufs=1)
        v_t = [v_pool.tile([128, CL], f32r, tag=f"v{mt}", name=f"v{mt}")
               for mt in range(TT)]
        wv_all = wvpool.tile([128, KT, CL], bf16)
        # deferred issue (here, not at the top): keeps the HBM bandwidth
        # free for the stage-A-critical x chunk DMAs
        for k in range(KT):
            nc.scalar.dma_start(out=wv_all[:, k], in_=wv3[:, k])
        wv_t = [wv_all[:, k] for k in range(KT)]

        rope = [None] * 8
        rtmp = tc.alloc_tile_pool(name="rope_tmp", bufs=4)
        psr = tc.alloc_tile_pool(name="ps_rot", bufs=4, space="PSUM")
        ps2 = tc.alloc_tile_pool(name="ps_v", bufs=4, space="PSUM")

        def emit_v_pair(pair):
            for half in range(2):
                mt = 2 * pair + half
                tsl = slice(mt * 128, (mt + 1) * 128)
                ps = ps2.tile([128, CL], f32, tag="ps_v", name="ps_v")
                for k in range(KT):
                    nc.tensor.matmul(
                        ps, x_t[k][:, tsl], wv_t[k],
                        start=(k == 0), stop=(k == KT - 1),
                    )
                nc.scalar.copy(out=v_t[mt], in_=ps)

        for idx, m in enumerate((0, 4, 1, 5, 2, 6, 3, 7)):
            tmp = []
            for n in range(NT):
                sl = slice(n * TC, (n + 1) * TC)
                ps = psr.tile([128, TC], f32, tag="ps_rot", name="ps_rot")
                nc.tensor.matmul(ps, rt_sb, qkT[m][:, sl], start=True, stop=True)
                t1 = rtmp.tile([128, TC], bf16, tag="t1", name="t1")
                t2 = rtmp.tile([128, TC], bf16, tag="t2", name="t2")
                nc.vector.tensor_mul(t1, ps, sin_sb[:, sl])
                nc.vector.tensor_mul(t2, qkT[m][:, sl], cos_sb[:, sl])
                tmp.append((t1, t2))
            ro = qk_pool.tile([128, T], bf16, tag=f"qk{m}", name=f"rope{m}")
            for n in range(NT):
                sl = slice(n * TC, (n + 1) * TC)
                nc.vector.tensor_add(ro[:, sl], tmp[n][0], tmp[n][1])
            rope[m] = ro
            emit_v_pair(idx)
        ps2.release()
        psr.release()
        rtmp.release()
        wvpool.release()
        xpool.release()
        tpool.release()

        # ---- stage D attention (query-chunk outer) + stage E interleaved ----
        y_pool = tc.alloc_tile_pool(name="yT_sb", bufs=1)
        yT = [y_pool.tile([128, T], bf16, tag=f"yT{h}", name=f"yT{h}")
              for h in range(HPC)]
        wppool = tc.alloc_tile_pool(name="wp_sb", bufs=1)
        wp_all = wppool.tile([128, HPC, C], bf16)
        for hk in range(HPC):
            nc.scalar.dma_start(out=wp_all[:, hk], in_=wp3[:, hk])
        wp_t = [wp_all[:, hk] for hk in range(HPC)]

        pp_pool = tc.alloc_tile_pool(name="pp", bufs=6)
        sm_pool = tc.alloc_tile_pool(name="small", bufs=2)
        ps_s = tc.alloc_tile_pool(name="ps_s", bufs=3, space="PSUM")
        ps_y = tc.alloc_tile_pool(name="ps_y", bufs=2, space="PSUM")
        ps_o = tc.alloc_tile_pool(name="ps_o", bufs=1, space="PSUM")
        ps_d = tc.alloc_tile_pool(name="ps_d", bufs=2, space="PSUM")  # two halves
        opool = tc.alloc_tile_pool(name="ostage", bufs=2)

        ready_E = []     # mt values whose yT inputs are complete
        e_state = [None, 0]  # open [ot_tile, next_n] for current mt

        e_pools = [ps_o]
        e_ctr = [0]

        def emit_e_subgroup():
            # one (mt, n) block: 4 accumulating matmuls + copy to the
            # staged out row; DMA the full row after its 4th block.
            if e_state[0] is None:
                if not ready_E:
                    return
                e_state[0] = (ready_E.pop(0),
                              opool.tile([128, C], bf16, tag="ot", name="ot"))
                e_state[1] = 0
            mt, ot = e_state[0]
            n = e_state[1]
            msl = slice(mt * 128, (mt + 1) * 128)
            pool = e_pools[e_ctr[0] % len(e_pools)]
            e_ctr[0] += 1
            ps = pool.tile([128, TC], f32, tag="o", name="o_ps")
            for hk in range(HPC):
                nc.tensor.matmul(
                    ps, yT[hk][:, msl], wp_t[hk][:, n * TC:(n + 1) * TC],
                    start=(hk == 0), stop=(hk == HPC - 1),
                )
            nc.vector.tensor_copy(out=ot[:, n * TC:(n + 1) * TC], in_=ps)
            if mt >= TT - 4:
                # final blocks ride the Activation DGE queue, which is
                # idle by then (exp work done) — no backlog behind the
                # interior rows on sync
                nc.scalar.dma_start(out=out[msl, n * TC:(n + 1) * TC],
                                    in_=ot[:, n * TC:(n + 1) * TC])
            e_state[1] += 1
            if e_state[1] == NT:
                if mt < TT - 4:
                    nc.sync.dma_start(out=out[msl, :], in_=ot)
                e_state[0] = None

        for I in range(NT):
            isl = slice(I * TC, (I + 1) * TC)
            jl = _JLISTS[I]
            for h in range(HPC):
                q_h = rope[h]
                k_h = rope[4 + h]
                y_ps = ps_y.tile([128, TC], f32, tag="y", name="y_ps")
                d_ps = ps_d.tile([1, TC], f32, tag="d", name="d_ps")
                pps = []

                def emit_S(jidx):
                    J, dmi = jl[jidx]
                    off = 0 if dmi is None else dmi * 128
                    osl = slice(off, TC)
                    s_ps = ps_s.tile([128, TC], f32, tag="s", name="s_ps")
                    nc.tensor.matmul(
                        s_ps[:, osl], k_h[:, J * 128:(J + 1) * 128],
                        q_h[:, I * TC + off:(I + 1) * TC],
                        start=True, stop=True,
                    )
                    pp = pp_pool.tile([128, TC], f32r, tag="pp", name="pp")
                    nc.scalar.activation(out=pp[:, osl], in_=s_ps[:, osl],
                                         func=Exp, scale=SCALE)
                    if dmi is not None:
                        # in-place masked multiply on the single 128x128
                        # triangle block at the streamed range's head
                        dsl = slice(off, off + 128)
                        nc.vector.tensor_mul(pp[:, dsl], pp[:, dsl], tri_sb)
                    pps.append((pp, osl, J, dmi))

                emit_S(0)
                for jidx in range(len(jl)):
                    if jidx + 1 < len(jl):
                        emit_S(jidx + 1)
                    pp, osl, J, dmi = pps[jidx]
                    if dmi is not None:
                        # extra PE fill while the DVE mask-mul lands
                        emit_e_subgroup()
                    first = jidx == 0
                    last = jidx == len(jl) - 1
                    nc.tensor.matmul(
                        y_ps[:, osl], v_t[J][:, h * 128:(h + 1) * 128],
                        pp[:, osl], start=first, stop=last,
                    )
                    nc.tensor.matmul(d_ps[:, osl], ones_sb, pp[:, osl],
                                     start=first, stop=last)
                    if jidx % 2 == 1 or len(ready_E) > 8:
                        emit_e_subgroup()
                recip = sm_pool.tile([1, TC], f32, tag="recip", name="recip")
                nc.vector.reciprocal_approx_fast(out=recip, in_=d_ps)
                recipB = sm_pool.tile([128, TC], f32, tag="recipB", name="recipB")
                nc.gpsimd.partition_broadcast(recipB, recip)
                nc.vector.tensor_mul(yT[h][:, isl], y_ps, recipB)
            # all heads' columns for chunk I are now complete
            ready_E.extend(range(4 * I, 4 * I + 4))
        ps_d.release()
        ps_o2 = tc.alloc_tile_pool(name="ps_o2", bufs=1, space="PSUM")
        e_pools.append(ps_o2)
        while ready_E or e_state[0] is not None:
            emit_e_subgroup()

        for p in (opool, sm_pool, pp_pool, wppool, y_pool, v_pool,
                  qk_pool, mpool, ps_o2, ps_o, ps_y, ps_s):
            p.release()
    nc.compile()
    return nc


def _host_prep(x, w_qkv, w_proj, freqs_cis):
    """Build per-core input maps (slicing + layout prep only)."""
    try:
        import ml_dtypes
        bf = ml_dtypes.bfloat16
    except ImportError:
        import jax.numpy as jnp
        bf = jnp.bfloat16
    x = np.asarray(x, dtype=np.float32)
    w_qkv = np.asarray(w_qkv, dtype=np.float32)
    w_proj = np.asarray(w_proj, dtype=np.float32)
    fc = np.asarray(freqs_cis, dtype=np.float32)

    xTb = [np.ascontiguousarray(x[b].T).astype(bf) for b in range(B)]

    cos = fc[:, :, 0].T  # [64, T]
    sin = fc[:, :, 1].T
    cosP = np.repeat(cos, 2, axis=0).astype(bf)  # [128, T]
    sinP = np.repeat(sin, 2, axis=0).astype(bf)

    rt = np.zeros((HD, HD), dtype=np.float32)
    for d in range(HD // 2):
        rt[2 * d, 2 * d + 1] = 1.0
        rt[2 * d + 1, 2 * d] = -1.0
    rt = rt.astype(bf)

    ii = np.arange(128)[None, :]
    jj = np.arange(128)[:, None]
    tri = (ii >= jj).astype(np.float32)

    ones = np.ones((128, 1), dtype=np.float32)

    in_maps = []
    for core in range(8):
        b = core // 4
        g = core % 4
        qc = w_qkv[:, 512 * g: 512 * (g + 1)]
        kc = w_qkv[:, 2048 + 512 * g: 2048 + 512 * (g + 1)]
        vc = np.ascontiguousarray(w_qkv[:, 4096 + 512 * g: 4096 + 512 * (g + 1)]).astype(bf)
        wqk_c = np.concatenate([qc, kc], axis=1).astype(bf)
        wp_c = np.ascontiguousarray(w_proj[512 * g: 512 * (g + 1), :]).astype(bf)
        in_maps.append({
            "xT": xTb[b],
            "wqk": wqk_c,
            "wv": vc,
            "wp": wp_c,
            "cosP": cosP,
            "sinP": sinP,
            "rt": rt,
            "tri": tri,
            "ones": ones,
        })
    return in_maps


def _get_nc():
    if "nc" not in _CACHE:
        _CACHE["nc"] = _build_nc()
    return _CACHE["nc"]


def kernel(x, w_qkv, w_proj, freqs_cis, attn_mask, _trace=False):
    from concourse.bass_utils import run_bass_kernel_spmd

    in_maps = _host_prep(x, w_qkv, w_proj, freqs_cis)
    nc = _get_nc()
    res = run_bass_kernel_spmd(
        nc, in_maps, core_ids=list(range(8)), trace=_trace,
    )
    outs = [np.asarray(r["out"]).astype(np.float64) for r in res.results]
    full = np.stack([
        outs[0] + outs[1] + outs[2] + outs[3],
        outs[4] + outs[5] + outs[6] + outs[7],
    ]).astype(np.float32)
    if _trace:
        kernel._last_results = res
    return full


# revision 12
# speedup vs baseline: 1.1953x; 1.0197x over previous
"""Trainium2 Bass kernel for prefix-LM CausalSelfAttention.

Problem: B=2, T=2048, C=2048, H=16 heads (hd=128), prefix-LM mask
(bidirectional over first half, causal after), RoPE on q/k.

Sharding over 8 cores: data-parallel on batch (2) x tensor-parallel on
heads (4 heads per core). Each core computes a partial output projection
(its heads' contribution); partials (bf16) are summed on host.

Weight/activation matmul operands are bf16: bf16 stationary operands get
fast-weight-load (83ns vs 173ns f32r). Softmax probabilities stay f32r
(ACT-written bf16 streams ~40% slower as the MOVING matmul operand;
f32r moving with free dim >= 256 runs at 1 row/cycle). PSUM is f32.

Per-core dataflow:
  A. qT/kT = W_{q,k}^T @ x^T   [hd*4, T] transposed layout (head-major).
     DMAs are consolidated (one 3D DMA for all w k-tiles, one per x
     chunk) and split across the Sync and Activation DGE queues.
     PE warm-up matmuls run during the head DMA wait (p-state ramp).
  B. RoPE via pair-swap permutation matmul + DVE combine, interleaved
     with (C) so the PE stays fed while DVE does the rope math
  C. v = x @ Wv   [T, hd*4] natural layout (x tiles as stationary);
     v stays f32r (walrus rejects bf16-stationary x f32r-moving mixes)
  D. attention, query-chunk outer / head inner, software-pipelined:
     S(j+1) issues before PV(j) so the exp latency hides behind PE
     work. Per 512-wide chunk I:
       S'[J] = k_rope[:,J]^T-tile x q_rope[:,I]    (scores transposed)
       P'[J] = exp(S' / sqrt(hd))                  (ACT, PSUM->SBUF f32r)
       diagonal tiles: in-place DVE multiply of the single 128x128
       triangle block (mask is the same triangle for every diag tile)
       y_psum += v[J,h]^T-as-lhsT x P'[J]
       d_psum += ones^T x P'[J]                    (denominator)
     normalize y^T by broadcast fast-reciprocal of d
  E. partial out = yT^T-as-lhsT @ Wp; interleaved into the NEXT query
     chunk's attention stream; rows staged in SBUF as bf16 and DMA'd
     per 128-token row block (halves HBM write traffic).

Fully-masked key tiles are skipped (structural sparsity: 44/64 tiles/head).
"""
import math

import numpy as np

N_HEAD = 16
B = 2
T = 2048
C = 2048
HD = 128
HPC = 4          # heads per core
CL = HPC * HD    # local C = 512
TC = 512         # chunk width (matmul moving free dim / psum bank)
NT = T // TC     # 4 chunks
KT = C // 128    # 16 contraction tiles over C
TT = T // 128    # 16 T tiles
SCALE = 1.0 / math.sqrt(HD)

# Per query-chunk I: list of (J, mask_idx) key tiles to compute.
_JLISTS = {
    0: [(j, None) for j in range(8)],
    1: [(j, None) for j in range(8)],
    2: [(j, None) for j in range(8)] + [(8 + d, d) for d in range(4)],
    3: [(j, None) for j in range(12)] + [(12 + d, d) for d in range(4)],
}

_CACHE = {}


def _build_nc():
    import concourse.tile as tile
    import concourse.mybir as mybir
    from concourse import bacc

    f32 = mybir.dt.float32
    bf16 = mybir.dt.bfloat16

    nc = bacc.Bacc(None, target_bir_lowering=False)

    xT = nc.dram_tensor("xT", [C, T], bf16, kind="ExternalInput")
    wqk = nc.dram_tensor("wqk", [C, 2 * CL], bf16, kind="ExternalInput")
    wv = nc.dram_tensor("wv", [C, CL], bf16, kind="ExternalInput")
    wp = nc.dram_tensor("wp", [CL, C], bf16, kind="ExternalInput")
    cosP = nc.dram_tensor("cosP", [HD, T], bf16, kind="ExternalInput")
    sinP = nc.dram_tensor("sinP", [HD, T], bf16, kind="ExternalInput")
    f32r = mybir.dt.float32r
    rt = nc.dram_tensor("rt", [HD, HD], bf16, kind="ExternalInput")
    tri = nc.dram_tensor("tri", [128, 128], f32r, kind="ExternalInput")
    ones = nc.dram_tensor("ones", [128, 1], f32r, kind="ExternalInput")
    out = nc.dram_tensor("out", [T, C], bf16, kind="ExternalOutput")

    xT3 = xT.rearrange("(kt p) t -> p kt t", p=128)
    wqk3 = wqk.rearrange("(kt p) m -> p kt m", p=128)
    wv3 = wv.rearrange("(kt p) m -> p kt m", p=128)
    wp3 = wp.rearrange("(kt p) m -> p kt m", p=128)

    Exp = mybir.ActivationFunctionType.Exp

    with tile.TileContext(nc) as tc:
        mpool = tc.alloc_tile_pool(name="misc", bufs=1)
        qk_pool = tc.alloc_tile_pool(name="qkrope", bufs=1)
        tpool = tc.alloc_tile_pool(name="trig", bufs=1, side="right")
        xpool = tc.alloc_tile_pool(name="xt_sb", bufs=1, side="right")

        rt_sb = mpool.tile([HD, HD], bf16)
        ones_sb = mpool.tile([128, 1], f32r)
        tri_sb = mpool.tile([128, 128], f32r)
        cos_sb = tpool.tile([HD, T], bf16)
        sin_sb = tpool.tile([HD, T], bf16)
        warm_sb = mpool.tile([1, 1], f32)

        qkT = [qk_pool.tile([128, T], bf16, tag=f"qk{m}", name=f"qk{m}") for m in range(8)]

        # ---- stage A: qT/kT = W_{q,k}^T @ x^T, head-major tiles ----
        # Consolidated DMAs: rt first (tiny, gates PE warm-up), then the
        # critical x chunk-0 + all-w 3D DMAs, then x chunks 1..3. The
        # Activation DGE queue carries everything stage B/C/E needs.
        wpool = tc.alloc_tile_pool(name="wqk_sb", bufs=1)
        ps1 = tc.alloc_tile_pool(name="ps_qk", bufs=8, space="PSUM")
        # Many small interleaved DMAs on the sync queue (baseline
        # pattern — measured fastest head; queue-splitting the reads
        # loses to it).
        nc.sync.dma_start(out=rt_sb, in_=rt[:, :])
        w_all = wpool.tile([128, KT, 2 * CL], bf16)
        x_all = xpool.tile([128, KT, T], bf16)
        for k in range(KT):
            nc.sync.dma_start(out=w_all[:, k], in_=wqk3[:, k])
            nc.sync.dma_start(out=x_all[:, k, 0:TC], in_=xT3[:, k, 0:TC])
        for k in range(KT):
            nc.sync.dma_start(out=x_all[:, k, TC:], in_=xT3[:, k, TC:])
        w_t = [w_all[:, k] for k in range(KT)]
        x_t = [x_all[:, k] for k in range(KT)]

        # secondary inputs on the Activation DGE queue (parallel to sync)
        nc.scalar.dma_start(out=cos_sb, in_=cosP[:, :])
        nc.scalar.dma_start(out=sin_sb, in_=sinP[:, :])
        nc.scalar.dma_start(out=ones_sb, in_=ones[:, :])
        nc.scalar.dma_start(out=tri_sb, in_=tri[:, :])

        # warm the ACT exp table during stage A (one-time ~2.7us load)
        nc.scalar.activation(out=warm_sb, in_=rt_sb[0:1, 0:1], func=Exp)
        # PE p-state warm-up: ~4us of dummy matmuls while the w/x DMAs
        # land. rt_sb arrives first (tiny DMA issued before the big ones).
        for wi in range(16):
            ps = ps1.tile([128, TC], f32, tag="ps_qk", name="warm")
            nc.tensor.matmul(ps[:, 0:HD], rt_sb, rt_sb, start=True, stop=True)

        for n in range(NT):
            nsl = slice(n * TC, (n + 1) * TC)
            for m in range(8):
                ps = ps1.tile([128, TC], f32, tag="ps_qk", name="ps_qk")
                for k in range(KT):
                    nc.tensor.matmul(
                        ps, w_t[k][:, m * 128:(m + 1) * 128], x_t[k][:, nsl],
                        start=(k == 0), stop=(k == KT - 1),
                    )
                nc.vector.tensor_copy(out=qkT[m][:, nsl], in_=ps)
        wpool.release()
        ps1.release()

        # ---- stage B+C interleaved: RoPE (PE tiny, DVE heavy) and
        # v = x @ Wv (PE heavy). Emitting v matmuls after each head's rope
        # keeps the PE busy while DVE works through the rope muls.
        v_pool = tc.alloc_tile_pool(name="v_sb", bufs=1)
        wvpool = tc.alloc_tile_pool(name="wv_sb", bufs=1)
        v_t = [v_pool.tile([128, CL], f32r, tag=f"v{mt}", name=f"v{mt}")
               for mt in range(TT)]
        wv_all = wvpool.tile([128, KT, CL], bf16)
        # deferred issue (here, not at the top): keeps the HBM bandwidth
        # free for the stage-A-critical x chunk DMAs
        for k in range(KT):
            nc.scalar.dma_start(out=wv_all[:, k], in_=wv3[:, k])
        wv_t = [wv_all[:, k] for k in range(KT)]

        rope = [None] * 8
        rtmp = tc.alloc_tile_pool(name="rope_tmp", bufs=4)
        psr = tc.alloc_tile_pool(name="ps_rot", bufs=4, space="PSUM")
        ps2 = tc.alloc_tile_pool(name="ps_v", bufs=4, space="PSUM")

        def emit_v_pair(pair):
            for half in range(2):
                mt = 2 * pair + half
                tsl = slice(mt * 128, (mt + 1) * 128)
                ps = ps2.tile([128, CL], f32, tag="ps_v", name="ps_v")
                for k in range(KT):
                    nc.tensor.matmul(
                        ps, x_t[k][:, tsl], wv_t[k],
                        start=(k == 0), stop=(k == KT - 1),
                    )
                nc.scalar.copy(out=v_t[mt], in_=ps)

        for idx, m in enumerate((0, 4, 1, 5, 2, 6, 3, 7)):
            tmp = []
            for n in range(NT):
                sl = slice(n * TC, (n + 1) * TC)
                ps = psr.tile([128, TC], f32, tag="ps_rot", name="ps_rot")
                nc.tensor.matmul(ps, rt_sb, qkT[m][:, sl], start=True, stop=True)
                t1 = rtmp.tile([128, TC], bf16, tag="t1", name="t1")
                t2 = rtmp.tile([128, TC], bf16, tag="t2", name="t2")
                nc.vector.tensor_mul(t1, ps, sin_sb[:, sl])
                nc.vector.tensor_mul(t2, qkT[m][:, sl], cos_sb[:, sl])
                tmp.append((t1, t2))
            ro = qk_pool.tile([128, T], bf16, tag=f"qk{m}", name=f"rope{m}")
            for n in range(NT):
                sl = slice(n * TC, (n + 1) * TC)
                nc.vector.tensor_add(ro[:, sl], tmp[n][0], tmp[n][1])
            rope[m] = ro
            emit_v_pair(idx)
        ps2.release()
        psr.release()
        rtmp.release()
        wvpool.release()
        xpool.release()
        tpool.release()

        # ---- stage D attention (query-chunk outer) + stage E interleaved ----
        y_pool = tc.alloc_tile_pool(name="yT_sb", bufs=1)
        yT = [y_pool.tile([128, T], bf16, tag=f"yT{h}", name=f"yT{h}")
              for h in range(HPC)]
        wppool = tc.alloc_tile_pool(name="wp_sb", bufs=1)
        wp_all = wppool.tile([128, HPC, C], bf16)
        for hk in range(HPC):
            nc.scalar.dma_start(out=wp_all[:, hk], in_=wp3[:, hk])
        wp_t = [wp_all[:, hk] for hk in range(HPC)]

        pp_pool = tc.alloc_tile_pool(name="pp", bufs=6)
        sm_pool = tc.alloc_tile_pool(name="small", bufs=2)
        ps_s = tc.alloc_tile_pool(name="ps_s", bufs=3, space="PSUM")
        ps_y = tc.alloc_tile_pool(name="ps_y", bufs=2, space="PSUM")
        ps_o = tc.alloc_tile_pool(name="ps_o", bufs=1, space="PSUM")
        ps_d = tc.alloc_tile_pool(name="ps_d", bufs=2, space="PSUM")  # two halves
        opool = tc.alloc_tile_pool(name="ostage", bufs=2)

        ready_E = []     # mt values whose yT inputs are complete
        e_state = [None, 0]  # open [ot_tile, next_n] for current mt

        e_pools = [ps_o]
        e_ctr = [0]

        def emit_e_subgroup():
            # one (mt, n) block: 4 accumulating matmuls + copy to the
            # staged out row; DMA the full row after its 4th block.
            if e_state[0] is None:
                if not ready_E:
                    return
                e_state[0] = (ready_E.pop(0),
                              opool.tile([128, C], bf16, tag="ot", name="ot"))
                e_state[1] = 0
            mt, ot = e_state[0]
            n = e_state[1]
            msl = slice(mt * 128, (mt + 1) * 128)
            pool = e_pools[e_ctr[0] % len(e_pools)]
            e_ctr[0] += 1
            ps = pool.tile([128, TC], f32, tag="o", name="o_ps")
            for hk in range(HPC):
                nc.tensor.matmul(
                    ps, yT[hk][:, msl], wp_t[hk][:, n * TC:(n + 1) * TC],
                    start=(hk == 0), stop=(hk == HPC - 1),
                )
            nc.vector.tensor_copy(out=ot[:, n * TC:(n + 1) * TC], in_=ps)
            if mt >= TT - 4:
                # final blocks alternate across both DGE queues (both
                # near-idle by then) so the very last chunk lands on an
                # empty queue
                eng = nc.scalar if (e_ctr[0] % 2) else nc.sync
                eng.dma_start(out=out[msl, n * TC:(n + 1) * TC],
                              in_=ot[:, n * TC:(n + 1) * TC])
            e_state[1] += 1
            if e_state[1] == NT:
                if mt < TT - 4:
                    nc.sync.dma_start(out=out[msl, :], in_=ot)
                e_state[0] = None

        for I in range(NT):
            isl = slice(I * TC, (I + 1) * TC)
            jl = _JLISTS[I]
            for h in range(HPC):
                q_h = rope[h]
                k_h = rope[4 + h]
                y_ps = ps_y.tile([128, TC], f32, tag="y", name="y_ps")
                d_ps = ps_d.tile([1, TC], f32, tag="d", name="d_ps")
                pps = []

                def emit_S(jidx):
                    J, dmi = jl[jidx]
                    off = 0 if dmi is None else dmi * 128
                    osl = slice(off, TC)
                    s_ps = ps_s.tile([128, TC], f32, tag="s", name="s_ps")
                    nc.tensor.matmul(
                        s_ps[:, osl], k_h[:, J * 128:(J + 1) * 128],
                        q_h[:, I * TC + off:(I + 1) * TC],
                        start=True, stop=True,
                    )
                    pp = pp_pool.tile([128, TC], f32r, tag="pp", name="pp")
                    nc.scalar.activation(out=pp[:, osl], in_=s_ps[:, osl],
                                         func=Exp, scale=SCALE)
                    if dmi is not None:
                        # in-place masked multiply on the single 128x128
                        # triangle block at the streamed range's head
                        dsl = slice(off, off + 128)
                        nc.vector.tensor_mul(pp[:, dsl], pp[:, dsl], tri_sb)
                    pps.append((pp, osl, J, dmi))

                emit_S(0)
                for jidx in range(len(jl)):
                    if jidx + 1 < len(jl):
                        emit_S(jidx + 1)
                    pp, osl, J, dmi = pps[jidx]
                    if dmi is not None:
                        # extra PE fill while the DVE mask-mul lands
                        emit_e_subgroup()
                    first = jidx == 0
                    last = jidx == len(jl) - 1
                    nc.tensor.matmul(
                        y_ps[:, osl], v_t[J][:, h * 128:(h + 1) * 128],
                        pp[:, osl], start=first, stop=last,
                    )
                    nc.tensor.matmul(d_ps[:, osl], ones_sb, pp[:, osl],
                                     start=first, stop=last)
                    if jidx % 2 == 1 or len(ready_E) > 8:
                        emit_e_subgroup()
                recip = sm_pool.tile([1, TC], f32, tag="recip", name="recip")
                nc.vector.reciprocal_approx_fast(out=recip, in_=d_ps)
                recipB = sm_pool.tile([128, TC], f32, tag="recipB", name="recipB")
                nc.gpsimd.partition_broadcast(recipB, recip)
                nc.vector.tensor_mul(yT[h][:, isl], y_ps, recipB)
            # all heads' columns for chunk I are now complete
            ready_E.extend(range(4 * I, 4 * I + 4))
        ps_d.release()
        ps_o2 = tc.alloc_tile_pool(name="ps_o2", bufs=1, space="PSUM")
        e_pools.append(ps_o2)
        while ready_E or e_state[0] is not None:
            emit_e_subgroup()

        for p in (opool, sm_pool, pp_pool, wppool, y_pool, v_pool,
                  qk_pool, mpool, ps_o2, ps_o, ps_y, ps_s):
            p.release()
    nc.compile()
    return nc


def _host_prep(x, w_qkv, w_proj, freqs_cis):
    """Build per-core input maps (slicing + layout prep only)."""
    try:
        import ml_dtypes
        bf = ml_dtypes.bfloat16
    except ImportError:
        import jax.numpy as jnp
        bf = jnp.bfloat16
    x = np.asarray(x, dtype=np.float32)
    w_qkv = np.asarray(w_qkv, dtype=np.float32)
    w_proj = np.asarray(w_proj, dtype=np.float32)
    fc = np.asarray(freqs_cis, dtype=np.float32)

    xTb = [np.ascontiguousarray(x[b].T).astype(bf) for b in range(B)]

    cos = fc[:, :, 0].T  # [64, T]
    sin = fc[:, :, 1].T
    cosP = np.repeat(cos, 2, axis=0).astype(bf)  # [128, T]
    sinP = np.repeat(sin, 2, axis=0).astype(bf)

    rt = np.zeros((HD, HD), dtype=np.float32)
    for d in range(HD // 2):
        rt[2 * d, 2 * d + 1] = 1.0
        rt[2 * d + 1, 2 * d] = -1.0
    rt = rt.astype(bf)

    ii = np.arange(128)[None, :]
    jj = np.arange(128)[:, None]
    tri = (ii >= jj).astype(np.float32)

    ones = np.ones((128, 1), dtype=np.float32)

    in_maps = []
    for core in range(8):
        b = core // 4
        g = core % 4
        qc = w_qkv[:, 512 * g: 512 * (g + 1)]
        kc = w_qkv[:, 2048 + 512 * g: 2048 + 512 * (g + 1)]
        vc = np.ascontiguousarray(w_qkv[:, 4096 + 512 * g: 4096 + 512 * (g + 1)]).astype(bf)
        wqk_c = np.concatenate([qc, kc], axis=1).astype(bf)
        wp_c = np.ascontiguousarray(w_proj[512 * g: 512 * (g + 1), :]).astype(bf)
        in_maps.append({
            "xT": xTb[b],
            "wqk": wqk_c,
            "wv": vc,
            "wp": wp_c,
            "cosP": cosP,
            "sinP": sinP,
            "rt": rt,
            "tri": tri,
            "ones": ones,
        })
    return in_maps


def _get_nc():
    if "nc" not in _CACHE:
        _CACHE["nc"] = _build_nc()
    return _CACHE["nc"]


def kernel(x, w_qkv, w_proj, freqs_cis, attn_mask, _trace=False):
    from concourse.bass_utils import run_bass_kernel_spmd

    in_maps = _host_prep(x, w_qkv, w_proj, freqs_cis)
    nc = _get_nc()
    res = run_bass_kernel_spmd(
        nc, in_maps, core_ids=list(range(8)), trace=_trace,
    )
    outs = [np.asarray(r["out"]).astype(np.float64) for r in res.results]
    full = np.stack([
        outs[0] + outs[1] + outs[2] + outs[3],
        outs[4] + outs[5] + outs[6] + outs[7],
    ]).astype(np.float32)
    if _trace:
        kernel._last_results = res
    return full


# revision 15
# speedup vs baseline: 1.1973x; 1.0017x over previous
"""Trainium2 Bass kernel for prefix-LM CausalSelfAttention.

Problem: B=2, T=2048, C=2048, H=16 heads (hd=128), prefix-LM mask
(bidirectional over first half, causal after), RoPE on q/k.

Sharding over 8 cores: data-parallel on batch (2) x tensor-parallel on
heads (4 heads per core). Each core computes a partial output projection
(its heads' contribution); partials (bf16) are summed on host.

Weight/activation matmul operands are bf16: bf16 stationary operands get
fast-weight-load (83ns vs 173ns f32r). Softmax probabilities stay f32r
(ACT-written bf16 streams ~40% slower as the MOVING matmul operand;
f32r moving with free dim >= 256 runs at 1 row/cycle). PSUM is f32.

Per-core dataflow:
  A. qT/kT = W_{q,k}^T @ x^T   [hd*4, T] transposed layout (head-major).
     DMAs are consolidated (one 3D DMA for all w k-tiles, one per x
     chunk) and split across the Sync and Activation DGE queues.
     PE warm-up matmuls run during the head DMA wait (p-state ramp).
  B. RoPE via pair-swap permutation matmul + DVE combine, interleaved
     with (C) so the PE stays fed while DVE does the rope math
  C. v = x @ Wv   [T, hd*4] natural layout (x tiles as stationary);
     v stays f32r (walrus rejects bf16-stationary x f32r-moving mixes)
  D. attention, query-chunk outer / head inner, software-pipelined:
     S(j+1) issues before PV(j) so the exp latency hides behind PE
     work. Per 512-wide chunk I:
       S'[J] = k_rope[:,J]^T-tile x q_rope[:,I]    (scores transposed)
       P'[J] = exp(S' / sqrt(hd))                  (ACT, PSUM->SBUF f32r)
       diagonal tiles: in-place DVE multiply of the single 128x128
       triangle block (mask is the same triangle for every diag tile)
       y_psum += v[J,h]^T-as-lhsT x P'[J]
       d_psum += ones^T x P'[J]                    (denominator)
     normalize y^T by broadcast fast-reciprocal of d
  E. partial out = yT^T-as-lhsT @ Wp; interleaved into the NEXT query
     chunk's attention stream; rows staged in SBUF as bf16 and DMA'd
     per 128-token row block (halves HBM write traffic).

Fully-masked key tiles are skipped (structural sparsity: 44/64 tiles/head).
"""
import math

import numpy as np

N_HEAD = 16
B = 2
T = 2048
C = 2048
HD = 128
HPC = 4          # heads per core
CL = HPC * HD    # local C = 512
TC = 512         # chunk width (matmul moving free dim / psum bank)
NT = T // TC     # 4 chunks
KT = C // 128    # 16 contraction tiles over C
TT = T // 128    # 16 T tiles
SCALE = 1.0 / math.sqrt(HD)

# Per query-chunk I: list of (J, mask_idx) key tiles to compute.
_JLISTS = {
    0: [(j, None) for j in range(8)],
    1: [(j, None) for j in range(8)],
    2: [(j, None) for j in range(8)] + [(8 + d, d) for d in range(4)],
    3: [(j, None) for j in range(12)] + [(12 + d, d) for d in range(4)],
}

_CACHE = {}


def _build_nc():
    import concourse.tile as tile
    import concourse.mybir as mybir
    from concourse import bacc

    f32 = mybir.dt.float32
    bf16 = mybir.dt.bfloat16

    nc = bacc.Bacc(None, target_bir_lowering=False)

    xT = nc.dram_tensor("xT", [C, T], bf16, kind="ExternalInput")
    wqk = nc.dram_tensor("wqk", [C, 2 * CL], bf16, kind="ExternalInput")
    wv = nc.dram_tensor("wv", [C, CL], bf16, kind="ExternalInput")
    wp = nc.dram_tensor("wp", [CL, C], bf16, kind="ExternalInput")
    cosP = nc.dram_tensor("cosP", [HD, T], bf16, kind="ExternalInput")
    sinP = nc.dram_tensor("sinP", [HD, T], bf16, kind="ExternalInput")
    f32r = mybir.dt.float32r
    rt = nc.dram_tensor("rt", [HD, HD], bf16, kind="ExternalInput")
    tri = nc.dram_tensor("tri", [128, 128], f32r, kind="ExternalInput")
    ones = nc.dram_tensor("ones", [128, 1], f32r, kind="ExternalInput")
    out = nc.dram_tensor("out", [T, C], bf16, kind="ExternalOutput")

    xT3 = xT.rearrange("(kt p) t -> p kt t", p=128)
    wqk3 = wqk.rearrange("(kt p) m -> p kt m", p=128)
    wv3 = wv.rearrange("(kt p) m -> p kt m", p=128)
    wp3 = wp.rearrange("(kt p) m -> p kt m", p=128)

    Exp = mybir.ActivationFunctionType.Exp

    with tile.TileContext(nc) as tc:
        mpool = tc.alloc_tile_pool(name="misc", bufs=1)
        qk_pool = tc.alloc_tile_pool(name="qkrope", bufs=1)
        tpool = tc.alloc_tile_pool(name="trig", bufs=1, side="right")
        xpool = tc.alloc_tile_pool(name="xt_sb", bufs=1, side="right")

        rt_sb = mpool.tile([HD, HD], bf16)
        ones_sb = mpool.tile([128, 1], f32r)
        tri_sb = mpool.tile([128, 128], f32r)
        cos_sb = tpool.tile([HD, T], bf16)
        sin_sb = tpool.tile([HD, T], bf16)
        warm_sb = mpool.tile([1, 1], f32)

        qkT = [qk_pool.tile([128, T], bf16, tag=f"qk{m}", name=f"qk{m}") for m in range(8)]

        # ---- stage A: qT/kT = W_{q,k}^T @ x^T, head-major tiles ----
        # Consolidated DMAs: rt first (tiny, gates PE warm-up), then the
        # critical x chunk-0 + all-w 3D DMAs, then x chunks 1..3. The
        # Activation DGE queue carries everything stage B/C/E needs.
        wpool = tc.alloc_tile_pool(name="wqk_sb", bufs=1)
        ps1 = tc.alloc_tile_pool(name="ps_qk", bufs=8, space="PSUM")
        # Many small interleaved DMAs on the sync queue, one tile per
        # k (baseline pattern — fine-grained DMA->matmul dependencies;
        # big shared tiles coarsen the tracking and stall stage A).
        w_t = []
        x_t = []
        for k in range(KT):
            wt = wpool.tile([128, 2 * CL], bf16, tag=f"w{k}", name=f"w{k}")
            nc.sync.dma_start(out=wt, in_=wqk3[:, k])
            w_t.append(wt)
            xt = xpool.tile([128, T], bf16, tag=f"x{k}", name=f"x{k}")
            nc.sync.dma_start(out=xt[:, 0:TC], in_=xT3[:, k, 0:TC])
            x_t.append(xt)
        for k in range(KT):
            nc.sync.dma_start(out=x_t[k][:, TC:], in_=xT3[:, k, TC:])
        nc.sync.dma_start(out=rt_sb, in_=rt[:, :])

        # secondary inputs on the Activation DGE queue (parallel to sync)
        nc.scalar.dma_start(out=cos_sb, in_=cosP[:, :])
        nc.scalar.dma_start(out=sin_sb, in_=sinP[:, :])
        nc.scalar.dma_start(out=ones_sb, in_=ones[:, :])
        nc.scalar.dma_start(out=tri_sb, in_=tri[:, :])

        # warm the ACT exp table during stage A (one-time ~2.7us load)
        nc.scalar.activation(out=warm_sb, in_=rt_sb[0:1, 0:1], func=Exp)

        for n in range(NT):
            nsl = slice(n * TC, (n + 1) * TC)
            for m in range(8):
                ps = ps1.tile([128, TC], f32, tag="ps_qk", name="ps_qk")
                for k in range(KT):
                    nc.tensor.matmul(
                        ps, w_t[k][:, m * 128:(m + 1) * 128], x_t[k][:, nsl],
                        start=(k == 0), stop=(k == KT - 1),
                    )
                nc.vector.tensor_copy(out=qkT[m][:, nsl], in_=ps)
        wpool.release()
        ps1.release()

        # ---- stage B+C interleaved: RoPE (PE tiny, DVE heavy) and
        # v = x @ Wv (PE heavy). Emitting v matmuls after each head's rope
        # keeps the PE busy while DVE works through the rope muls.
        v_pool = tc.alloc_tile_pool(name="v_sb", bufs=1)
        wvpool = tc.alloc_tile_pool(name="wv_sb", bufs=1)
        v_t = [v_pool.tile([128, CL], f32r, tag=f"v{mt}", name=f"v{mt}")
               for mt in range(TT)]
        # deferred issue (here, not at the top): keeps the HBM bandwidth
        # free for the stage-A-critical x chunk DMAs
        wv_t = []
        for k in range(KT):
            wt = wvpool.tile([128, CL], bf16, tag=f"wv{k}", name=f"wv{k}")
            nc.scalar.dma_start(out=wt, in_=wv3[:, k])
            wv_t.append(wt)

        rope = [None] * 8
        rtmp = tc.alloc_tile_pool(name="rope_tmp", bufs=4)
        psr = tc.alloc_tile_pool(name="ps_rot", bufs=4, space="PSUM")
        ps2 = tc.alloc_tile_pool(name="ps_v", bufs=4, space="PSUM")

        def emit_v_pair(pair):
            for half in range(2):
                mt = 2 * pair + half
                tsl = slice(mt * 128, (mt + 1) * 128)
                ps = ps2.tile([128, CL], f32, tag="ps_v", name="ps_v")
                for k in range(KT):
                    nc.tensor.matmul(
                        ps, x_t[k][:, tsl], wv_t[k],
                        start=(k == 0), stop=(k == KT - 1),
                    )
                nc.scalar.copy(out=v_t[mt], in_=ps)

        for idx, m in enumerate((0, 4, 1, 5, 2, 6, 3, 7)):
            tmp = []
            for n in range(NT):
                sl = slice(n * TC, (n + 1) * TC)
                ps = psr.tile([128, TC], f32, tag="ps_rot", name="ps_rot")
                nc.tensor.matmul(ps, rt_sb, qkT[m][:, sl], start=True, stop=True)
                t1 = rtmp.tile([128, TC], bf16, tag="t1", name="t1")
                t2 = rtmp.tile([128, TC], bf16, tag="t2", name="t2")
                nc.vector.tensor_mul(t1, ps, sin_sb[:, sl])
                nc.vector.tensor_mul(t2, qkT[m][:, sl], cos_sb[:, sl])
                tmp.append((t1, t2))
            ro = qk_pool.tile([128, T], bf16, tag=f"qk{m}", name=f"rope{m}")
            for n in range(NT):
                sl = slice(n * TC, (n + 1) * TC)
                nc.vector.tensor_add(ro[:, sl], tmp[n][0], tmp[n][1])
            rope[m] = ro
            emit_v_pair(idx)
        ps2.release()
        psr.release()
        rtmp.release()
        wvpool.release()
        xpool.release()
        tpool.release()

        # ---- stage D attention (query-chunk outer) + stage E interleaved ----
        y_pool = tc.alloc_tile_pool(name="yT_sb", bufs=1)
        yT = [y_pool.tile([128, T], bf16, tag=f"yT{h}", name=f"yT{h}")
              for h in range(HPC)]
        wppool = tc.alloc_tile_pool(name="wp_sb", bufs=1)
        wp_t = []
        for hk in range(HPC):
            wt = wppool.tile([128, C], bf16, tag=f"wp{hk}", name=f"wp{hk}")
            nc.scalar.dma_start(out=wt, in_=wp3[:, hk])
            wp_t.append(wt)

        pp_pool = tc.alloc_tile_pool(name="pp", bufs=6)
        sm_pool = tc.alloc_tile_pool(name="small", bufs=2)
        ps_s = tc.alloc_tile_pool(name="ps_s", bufs=3, space="PSUM")
        ps_y = tc.alloc_tile_pool(name="ps_y", bufs=2, space="PSUM")
        ps_o = tc.alloc_tile_pool(name="ps_o", bufs=1, space="PSUM")
        ps_d = tc.alloc_tile_pool(name="ps_d", bufs=2, space="PSUM")  # two halves
        opool = tc.alloc_tile_pool(name="ostage", bufs=2)

        ready_E = []     # mt values whose yT inputs are complete
        e_state = [None, 0]  # open [ot_tile, next_n] for current mt

        e_pools = [ps_o]
        e_ctr = [0]

        def emit_e_subgroup():
            # one (mt, n) block: 4 accumulating matmuls + copy to the
            # staged out row; DMA the full row after its 4th block.
            if e_state[0] is None:
                if not ready_E:
                    return
                e_state[0] = (ready_E.pop(0),
                              opool.tile([128, C], bf16, tag="ot", name="ot"))
                e_state[1] = 0
            mt, ot = e_state[0]
            n = e_state[1]
            msl = slice(mt * 128, (mt + 1) * 128)
            pool = e_pools[e_ctr[0] % len(e_pools)]
            e_ctr[0] += 1
            ps = pool.tile([128, TC], f32, tag="o", name="o_ps")
            for hk in range(HPC):
                nc.tensor.matmul(
                    ps, yT[hk][:, msl], wp_t[hk][:, n * TC:(n + 1) * TC],
                    start=(hk == 0), stop=(hk == HPC - 1),
                )
            nc.vector.tensor_copy(out=ot[:, n * TC:(n + 1) * TC], in_=ps)
            if mt >= TT - 4:
                # final blocks alternate across both DGE queues (both
                # near-idle by then) so the very last chunk lands on an
                # empty queue
                eng = nc.scalar if (e_ctr[0] % 2) else nc.sync
                eng.dma_start(out=out[msl, n * TC:(n + 1) * TC],
                              in_=ot[:, n * TC:(n + 1) * TC])
            e_state[1] += 1
            if e_state[1] == NT:
                if mt < TT - 4:
                    nc.sync.dma_start(out=out[msl, :], in_=ot)
                e_state[0] = None

        for I in range(NT):
            isl = slice(I * TC, (I + 1) * TC)
            jl = _JLISTS[I]
            for h in range(HPC):
                q_h = rope[h]
                k_h = rope[4 + h]
                y_ps = ps_y.tile([128, TC], f32, tag="y", name="y_ps")
                d_ps = ps_d.tile([1, TC], f32, tag="d", name="d_ps")
                pps = []

                def emit_S(jidx):
                    J, dmi = jl[jidx]
                    off = 0 if dmi is None else dmi * 128
                    osl = slice(off, TC)
                    s_ps = ps_s.tile([128, TC], f32, tag="s", name="s_ps")
                    nc.tensor.matmul(
                        s_ps[:, osl], k_h[:, J * 128:(J + 1) * 128],
                        q_h[:, I * TC + off:(I + 1) * TC],
                        start=True, stop=True,
                    )
                    pp = pp_pool.tile([128, TC], f32r, tag="pp", name="pp")
                    nc.scalar.activation(out=pp[:, osl], in_=s_ps[:, osl],
                                         func=Exp, scale=SCALE)
                    if dmi is not None:
                        # in-place masked multiply on the single 128x128
                        # triangle block at the streamed range's head
                        dsl = slice(off, off + 128)
                        nc.vector.tensor_mul(pp[:, dsl], pp[:, dsl], tri_sb)
                    pps.append((pp, osl, J, dmi))

                emit_S(0)
                for jidx in range(len(jl)):
                    if jidx + 1 < len(jl):
                        emit_S(jidx + 1)
                    pp, osl, J, dmi = pps[jidx]
                    if dmi is not None:
                        # extra PE fill while the DVE mask-mul lands
                        emit_e_subgroup()
                    first = jidx == 0
                    last = jidx == len(jl) - 1
                    nc.tensor.matmul(
                        y_ps[:, osl], v_t[J][:, h * 128:(h + 1) * 128],
                        pp[:, osl], start=first, stop=last,
                    )
                    nc.tensor.matmul(d_ps[:, osl], ones_sb, pp[:, osl],
                                     start=first, stop=last)
                    if jidx % 2 == 1 or len(ready_E) > 8:
                        emit_e_subgroup()
                recip = sm_pool.tile([1, TC], f32, tag="recip", name="recip")
                nc.vector.reciprocal_approx_fast(out=recip, in_=d_ps)
                recipB = sm_pool.tile([128, TC], f32, tag="recipB", name="recipB")
                nc.gpsimd.partition_broadcast(recipB, recip)
                nc.vector.tensor_mul(yT[h][:, isl], y_ps, recipB)
            # all heads' columns for chunk I are now complete
            ready_E.extend(range(4 * I, 4 * I + 4))
        ps_d.release()
        ps_o2 = tc.alloc_tile_pool(name="ps_o2", bufs=1, space="PSUM")
        e_pools.append(ps_o2)
        while ready_E or e_state[0] is not None:
            emit_e_subgroup()

        for p in (opool, sm_pool, pp_pool, wppool, y_pool, v_pool,
                  qk_pool, mpool, ps_o2, ps_o, ps_y, ps_s):
            p.release()
    nc.compile()
    return nc


def _host_prep(x, w_qkv, w_proj, freqs_cis):
    """Build per-core input maps (slicing + layout prep only)."""
    try:
        import ml_dtypes
        bf = ml_dtypes.bfloat16
    except ImportError:
        import jax.numpy as jnp
        bf = jnp.bfloat16
    x = np.asarray(x, dtype=np.float32)
    w_qkv = np.asarray(w_qkv, dtype=np.float32)
    w_proj = np.asarray(w_proj, dtype=np.float32)
    fc = np.asarray(freqs_cis, dtype=np.float32)

    xTb = [np.ascontiguousarray(x[b].T).astype(bf) for b in range(B)]

    cos = fc[:, :, 0].T  # [64, T]
    sin = fc[:, :, 1].T
    cosP = np.repeat(cos, 2, axis=0).astype(bf)  # [128, T]
    sinP = np.repeat(sin, 2, axis=0).astype(bf)

    rt = np.zeros((HD, HD), dtype=np.float32)
    for d in range(HD // 2):
        rt[2 * d, 2 * d + 1] = 1.0
        rt[2 * d + 1, 2 * d] = -1.0
    rt = rt.astype(bf)

    ii = np.arange(128)[None, :]
    jj = np.arange(128)[:, None]
    tri = (ii >= jj).astype(np.float32)

    ones = np.ones((128, 1), dtype=np.float32)

    in_maps = []
    for core in range(8):
        b = core // 4
        g = core % 4
        qc = w_qkv[:, 512 * g: 512 * (g + 1)]
        kc = w_qkv[:, 2048 + 512 * g: 2048 + 512 * (g + 1)]
        vc = np.ascontiguousarray(w_qkv[:, 4096 + 512 * g: 4096 + 512 * (g + 1)]).astype(bf)
        wqk_c = np.concatenate([qc, kc], axis=1).astype(bf)
        wp_c = np.ascontiguousarray(w_proj[512 * g: 512 * (g + 1), :]).astype(bf)
        in_maps.append({
            "xT": xTb[b],
            "wqk": wqk_c,
            "wv": vc,
            "wp": wp_c,
            "cosP": cosP,
            "sinP": sinP,
            "rt": rt,
            "tri": tri,
            "ones": ones,
        })
    return in_maps


def _get_nc():
    if "nc" not in _CACHE:
        _CACHE["nc"] = _build_nc()
    return _CACHE["nc"]


def kernel(x, w_qkv, w_proj, freqs_cis, attn_mask, _trace=False):
    from concourse.bass_utils import run_bass_kernel_spmd

    in_maps = _host_prep(x, w_qkv, w_proj, freqs_cis)
    nc = _get_nc()
    res = run_bass_kernel_spmd(
        nc, in_maps, core_ids=list(range(8)), trace=_trace,
    )
    outs = [np.asarray(r["out"]).astype(np.float64) for r in res.results]
    full = np.stack([
        outs[0] + outs[1] + outs[2] + outs[3],
        outs[4] + outs[5] + outs[6] + outs[7],
    ]).astype(np.float32)
    if _trace:
        kernel._last_results = res
    return full


# revision 17
# speedup vs baseline: 1.2025x; 1.0043x over previous
"""Trainium2 Bass kernel for prefix-LM CausalSelfAttention.

Problem: B=2, T=2048, C=2048, H=16 heads (hd=128), prefix-LM mask
(bidirectional over first half, causal after), RoPE on q/k.

Sharding over 8 cores: data-parallel on batch (2) x tensor-parallel on
heads (4 heads per core). Each core computes a partial output projection
(its heads' contribution); partials (bf16) are summed on host.

Weight/activation matmul operands are bf16: bf16 stationary operands get
fast-weight-load (83ns vs 173ns f32r). Softmax probabilities stay f32r
(ACT-written bf16 streams ~40% slower as the MOVING matmul operand;
f32r moving with free dim >= 256 runs at 1 row/cycle). PSUM is f32.

Per-core dataflow:
  A. qT/kT = W_{q,k}^T @ x^T   [hd*4, T] transposed layout (head-major).
     DMAs are consolidated (one 3D DMA for all w k-tiles, one per x
     chunk) and split across the Sync and Activation DGE queues.
     PE warm-up matmuls run during the head DMA wait (p-state ramp).
  B. RoPE via pair-swap permutation matmul + DVE combine, interleaved
     with (C) so the PE stays fed while DVE does the rope math
  C. v = x @ Wv   [T, hd*4] natural layout (x tiles as stationary);
     v stays f32r (walrus rejects bf16-stationary x f32r-moving mixes)
  D. attention, query-chunk outer / head inner, software-pipelined:
     S(j+1) issues before PV(j) so the exp latency hides behind PE
     work. Per 512-wide chunk I:
       S'[J] = k_rope[:,J]^T-tile x q_rope[:,I]    (scores transposed)
       P'[J] = exp(S' / sqrt(hd))                  (ACT, PSUM->SBUF f32r)
       diagonal tiles: in-place DVE multiply of the single 128x128
       triangle block (mask is the same triangle for every diag tile)
       y_psum += v[J,h]^T-as-lhsT x P'[J]
       d_psum += ones^T x P'[J]                    (denominator)
     normalize y^T by broadcast fast-reciprocal of d
  E. partial out = yT^T-as-lhsT @ Wp; interleaved into the NEXT query
     chunk's attention stream; rows staged in SBUF as bf16 and DMA'd
     per 128-token row block (halves HBM write traffic).

Fully-masked key tiles are skipped (structural sparsity: 44/64 tiles/head).
"""
import math

import numpy as np

N_HEAD = 16
B = 2
T = 2048
C = 2048
HD = 128
HPC = 4          # heads per core
CL = HPC * HD    # local C = 512
TC = 512         # chunk width (matmul moving free dim / psum bank)
NT = T // TC     # 4 chunks
KT = C // 128    # 16 contraction tiles over C
TT = T // 128    # 16 T tiles
SCALE = 1.0 / math.sqrt(HD)

# Per query-chunk I: list of (J, mask_idx) key tiles to compute.
_JLISTS = {
    0: [(j, None) for j in range(8)],
    1: [(j, None) for j in range(8)],
    2: [(j, None) for j in range(8)] + [(8 + d, d) for d in range(4)],
    3: [(j, None) for j in range(12)] + [(12 + d, d) for d in range(4)],
}

_CACHE = {}


def _build_nc():
    import concourse.tile as tile
    import concourse.mybir as mybir
    from concourse import bacc

    f32 = mybir.dt.float32
    bf16 = mybir.dt.bfloat16

    nc = bacc.Bacc(None, target_bir_lowering=False)

    xT = nc.dram_tensor("xT", [C, T], bf16, kind="ExternalInput")
    wqk = nc.dram_tensor("wqk", [C, 2 * CL], bf16, kind="ExternalInput")
    wv = nc.dram_tensor("wv", [C, CL], bf16, kind="ExternalInput")
    wp = nc.dram_tensor("wp", [CL, C], bf16, kind="ExternalInput")
    cosP = nc.dram_tensor("cosP", [HD, T], bf16, kind="ExternalInput")
    sinP = nc.dram_tensor("sinP", [HD, T], bf16, kind="ExternalInput")
    f32r = mybir.dt.float32r
    rt = nc.dram_tensor("rt", [HD, HD], bf16, kind="ExternalInput")
    tri = nc.dram_tensor("tri", [128, 128], f32r, kind="ExternalInput")
    ones = nc.dram_tensor("ones", [128, 1], f32r, kind="ExternalInput")
    out = nc.dram_tensor("out", [T, C], bf16, kind="ExternalOutput")

    xT3 = xT.rearrange("(kt p) t -> p kt t", p=128)
    wqk3 = wqk.rearrange("(kt p) m -> p kt m", p=128)
    wv3 = wv.rearrange("(kt p) m -> p kt m", p=128)
    wp3 = wp.rearrange("(kt p) m -> p kt m", p=128)

    Exp = mybir.ActivationFunctionType.Exp

    with tile.TileContext(nc) as tc:
        mpool = tc.alloc_tile_pool(name="misc", bufs=1)
        qk_pool = tc.alloc_tile_pool(name="qkrope", bufs=1)
        tpool = tc.alloc_tile_pool(name="trig", bufs=1, side="right")
        xpool = tc.alloc_tile_pool(name="xt_sb", bufs=1, side="right")

        rt_sb = mpool.tile([HD, HD], bf16)
        ones_sb = mpool.tile([128, 1], f32r)
        tri_sb = mpool.tile([128, 128], f32r)
        cos_sb = tpool.tile([HD, T], bf16)
        sin_sb = tpool.tile([HD, T], bf16)
        warm_sb = mpool.tile([1, 1], f32)

        qkT = [qk_pool.tile([128, T], bf16, tag=f"qk{m}", name=f"qk{m}") for m in range(8)]

        # ---- stage A: qT/kT = W_{q,k}^T @ x^T, head-major tiles ----
        # Consolidated DMAs: rt first (tiny, gates PE warm-up), then the
        # critical x chunk-0 + all-w 3D DMAs, then x chunks 1..3. The
        # Activation DGE queue carries everything stage B/C/E needs.
        wpool = tc.alloc_tile_pool(name="wqk_sb", bufs=1)
        ps1 = tc.alloc_tile_pool(name="ps_qk", bufs=8, space="PSUM")
        # Many small interleaved DMAs on the sync queue, one tile per
        # k (baseline pattern — fine-grained DMA->matmul dependencies;
        # big shared tiles coarsen the tracking and stall stage A).
        w_t = []
        x_t = []
        for k in range(KT):
            wt = wpool.tile([128, 2 * CL], bf16, tag=f"w{k}", name=f"w{k}")
            nc.sync.dma_start(out=wt, in_=wqk3[:, k])
            w_t.append(wt)
            xt = xpool.tile([128, T], bf16, tag=f"x{k}", name=f"x{k}")
            nc.sync.dma_start(out=xt[:, 0:TC], in_=xT3[:, k, 0:TC])
            x_t.append(xt)
        for k in range(KT):
            nc.sync.dma_start(out=x_t[k][:, TC:], in_=xT3[:, k, TC:])
        # trig after the critical reads on the same queue: concurrent
        # queues contend for HBM and stall the first stage-A chains
        nc.sync.dma_start(out=rt_sb, in_=rt[:, :])
        nc.sync.dma_start(out=cos_sb, in_=cosP[:, :])
        nc.sync.dma_start(out=sin_sb, in_=sinP[:, :])

        # warm the ACT exp table during stage A (one-time ~2.7us load)
        nc.scalar.activation(out=warm_sb, in_=rt_sb[0:1, 0:1], func=Exp)

        for n in range(NT):
            nsl = slice(n * TC, (n + 1) * TC)
            for m in range(8):
                ps = ps1.tile([128, TC], f32, tag="ps_qk", name="ps_qk")
                for k in range(KT):
                    nc.tensor.matmul(
                        ps, w_t[k][:, m * 128:(m + 1) * 128], x_t[k][:, nsl],
                        start=(k == 0), stop=(k == KT - 1),
                    )
                nc.vector.tensor_copy(out=qkT[m][:, nsl], in_=ps)
        wpool.release()
        ps1.release()

        # ---- stage B+C interleaved: RoPE (PE tiny, DVE heavy) and
        # v = x @ Wv (PE heavy). Emitting v matmuls after each head's rope
        # keeps the PE busy while DVE works through the rope muls.
        v_pool = tc.alloc_tile_pool(name="v_sb", bufs=1)
        wvpool = tc.alloc_tile_pool(name="wv_sb", bufs=1)
        v_t = [v_pool.tile([128, CL], f32r, tag=f"v{mt}", name=f"v{mt}")
               for mt in range(TT)]
        # deferred issue (here, not at the top): keeps the HBM bandwidth
        # free for the stage-A-critical x chunk DMAs
        wv_t = []
        for k in range(KT):
            wt = wvpool.tile([128, CL], bf16, tag=f"wv{k}", name=f"wv{k}")
            nc.scalar.dma_start(out=wt, in_=wv3[:, k])
            wv_t.append(wt)

        rope = [None] * 8
        rtmp = tc.alloc_tile_pool(name="rope_tmp", bufs=4)
        psr = tc.alloc_tile_pool(name="ps_rot", bufs=4, space="PSUM")
        ps2 = tc.alloc_tile_pool(name="ps_v", bufs=4, space="PSUM")

        def emit_v_pair(pair):
            for half in range(2):
                mt = 2 * pair + half
                tsl = slice(mt * 128, (mt + 1) * 128)
                ps = ps2.tile([128, CL], f32, tag="ps_v", name="ps_v")
                for k in range(KT):
                    nc.tensor.matmul(
                        ps, x_t[k][:, tsl], wv_t[k],
                        start=(k == 0), stop=(k == KT - 1),
                    )
                nc.scalar.copy(out=v_t[mt], in_=ps)

        for idx, m in enumerate((0, 4, 1, 5, 2, 6, 3, 7)):
            tmp = []
            for n in range(NT):
                sl = slice(n * TC, (n + 1) * TC)
                ps = psr.tile([128, TC], f32, tag="ps_rot", name="ps_rot")
                nc.tensor.matmul(ps, rt_sb, qkT[m][:, sl], start=True, stop=True)
                t1 = rtmp.tile([128, TC], bf16, tag="t1", name="t1")
                t2 = rtmp.tile([128, TC], bf16, tag="t2", name="t2")
                nc.vector.tensor_mul(t1, ps, sin_sb[:, sl])
                nc.vector.tensor_mul(t2, qkT[m][:, sl], cos_sb[:, sl])
                tmp.append((t1, t2))
            ro = qk_pool.tile([128, T], bf16, tag=f"qk{m}", name=f"rope{m}")
            for n in range(NT):
                sl = slice(n * TC, (n + 1) * TC)
                nc.vector.tensor_add(ro[:, sl], tmp[n][0], tmp[n][1])
            rope[m] = ro
            emit_v_pair(idx)
        ps2.release()
        psr.release()
        rtmp.release()
        wvpool.release()
        xpool.release()
        tpool.release()

        # ---- stage D attention (query-chunk outer) + stage E interleaved ----
        y_pool = tc.alloc_tile_pool(name="yT_sb", bufs=1)
        yT = [y_pool.tile([128, T], bf16, tag=f"yT{h}", name=f"yT{h}")
              for h in range(HPC)]
        wppool = tc.alloc_tile_pool(name="wp_sb", bufs=1)
        wp_t = []
        for hk in range(HPC):
            wt = wppool.tile([128, C], bf16, tag=f"wp{hk}", name=f"wp{hk}")
            nc.scalar.dma_start(out=wt, in_=wp3[:, hk])
            wp_t.append(wt)
        nc.scalar.dma_start(out=ones_sb, in_=ones[:, :])
        nc.scalar.dma_start(out=tri_sb, in_=tri[:, :])

        pp_pool = tc.alloc_tile_pool(name="pp", bufs=6)
        sm_pool = tc.alloc_tile_pool(name="small", bufs=2)
        ps_s = tc.alloc_tile_pool(name="ps_s", bufs=3, space="PSUM")
        ps_y = tc.alloc_tile_pool(name="ps_y", bufs=2, space="PSUM")
        ps_o = tc.alloc_tile_pool(name="ps_o", bufs=1, space="PSUM")
        ps_d = tc.alloc_tile_pool(name="ps_d", bufs=2, space="PSUM")  # two halves
        opool = tc.alloc_tile_pool(name="ostage", bufs=2)

        ready_E = []     # mt values whose yT inputs are complete
        e_state = [None, 0]  # open [ot_tile, next_n] for current mt

        e_pools = [ps_o]
        e_ctr = [0]

        def emit_e_subgroup():
            # one (mt, n) block: 4 accumulating matmuls + copy to the
            # staged out row; DMA the full row after its 4th block.
            if e_state[0] is None:
                if not ready_E:
                    return
                e_state[0] = (ready_E.pop(0),
                              opool.tile([128, C], bf16, tag="ot", name="ot"))
                e_state[1] = 0
            mt, ot = e_state[0]
            n = e_state[1]
            msl = slice(mt * 128, (mt + 1) * 128)
            pool = e_pools[e_ctr[0] % len(e_pools)]
            e_ctr[0] += 1
            ps = pool.tile([128, TC], f32, tag="o", name="o_ps")
            for hk in range(HPC):
                nc.tensor.matmul(
                    ps, yT[hk][:, msl], wp_t[hk][:, n * TC:(n + 1) * TC],
                    start=(hk == 0), stop=(hk == HPC - 1),
                )
            nc.vector.tensor_copy(out=ot[:, n * TC:(n + 1) * TC], in_=ps)
            if mt >= TT - 4:
                # final blocks alternate across both DGE queues (both
                # near-idle by then) so the very last chunk lands on an
                # empty queue
                eng = nc.scalar if (e_ctr[0] % 2) else nc.sync
                eng.dma_start(out=out[msl, n * TC:(n + 1) * TC],
                              in_=ot[:, n * TC:(n + 1) * TC])
            e_state[1] += 1
            if e_state[1] == NT:
                if mt < TT - 4:
                    nc.sync.dma_start(out=out[msl, :], in_=ot)
                e_state[0] = None

        for I in range(NT):
            isl = slice(I * TC, (I + 1) * TC)
            jl = _JLISTS[I]
            for h in range(HPC):
                q_h = rope[h]
                k_h = rope[4 + h]
                y_ps = ps_y.tile([128, TC], f32, tag="y", name="y_ps")
                d_ps = ps_d.tile([1, TC], f32, tag="d", name="d_ps")
                pps = []

                def emit_S(jidx):
                    J, dmi = jl[jidx]
                    off = 0 if dmi is None else dmi * 128
                    osl = slice(off, TC)
                    s_ps = ps_s.tile([128, TC], f32, tag="s", name="s_ps")
                    nc.tensor.matmul(
                        s_ps[:, osl], k_h[:, J * 128:(J + 1) * 128],
                        q_h[:, I * TC + off:(I + 1) * TC],
                        start=True, stop=True,
                    )
                    pp = pp_pool.tile([128, TC], f32r, tag="pp", name="pp")
                    nc.scalar.activation(out=pp[:, osl], in_=s_ps[:, osl],
                                         func=Exp, scale=SCALE)
                    if dmi is not None:
                        # in-place masked multiply on the single 128x128
                        # triangle block at the streamed range's head
                        dsl = slice(off, off + 128)
                        nc.vector.tensor_mul(pp[:, dsl], pp[:, dsl], tri_sb)
                    pps.append((pp, osl, J, dmi))

                emit_S(0)
                for jidx in range(len(jl)):
                    if jidx + 1 < len(jl):
                        emit_S(jidx + 1)
                    pp, osl, J, dmi = pps[jidx]
                    if dmi is not None:
                        # extra PE fill while the DVE mask-mul lands
                        emit_e_subgroup()
                    first = jidx == 0
                    last = jidx == len(jl) - 1
                    nc.tensor.matmul(
                        y_ps[:, osl], v_t[J][:, h * 128:(h + 1) * 128],
                        pp[:, osl], start=first, stop=last,
                    )
                    nc.tensor.matmul(d_ps[:, osl], ones_sb, pp[:, osl],
                                     start=first, stop=last)
                    if jidx % 2 == 1 or len(ready_E) > 8:
                        emit_e_subgroup()
                recip = sm_pool.tile([1, TC], f32, tag="recip", name="recip")
                nc.vector.reciprocal_approx_fast(out=recip, in_=d_ps)
                recipB = sm_pool.tile([128, TC], f32, tag="recipB", name="recipB")
                nc.gpsimd.partition_broadcast(recipB, recip)
                nc.vector.tensor_mul(yT[h][:, isl], y_ps, recipB)
            # all heads' columns for chunk I are now complete
            ready_E.extend(range(4 * I, 4 * I + 4))
        ps_d.release()
        ps_o2 = tc.alloc_tile_pool(name="ps_o2", bufs=1, space="PSUM")
        e_pools.append(ps_o2)
        while ready_E or e_state[0] is not None:
            emit_e_subgroup()

        for p in (opool, sm_pool, pp_pool, wppool, y_pool, v_pool,
                  qk_pool, mpool, ps_o2, ps_o, ps_y, ps_s):
            p.release()
    nc.compile()
    return nc


def _host_prep(x, w_qkv, w_proj, freqs_cis):
    """Build per-core input maps (slicing + layout prep only)."""
    try:
        import ml_dtypes
        bf = ml_dtypes.bfloat16
    except ImportError:
        import jax.numpy as jnp
        bf = jnp.bfloat16
    x = np.asarray(x, dtype=np.float32)
    w_qkv = np.asarray(w_qkv, dtype=np.float32)
    w_proj = np.asarray(w_proj, dtype=np.float32)
    fc = np.asarray(freqs_cis, dtype=np.float32)

    xTb = [np.ascontiguousarray(x[b].T).astype(bf) for b in range(B)]

    cos = fc[:, :, 0].T  # [64, T]
    sin = fc[:, :, 1].T
    cosP = np.repeat(cos, 2, axis=0).astype(bf)  # [128, T]
    sinP = np.repeat(sin, 2, axis=0).astype(bf)

    rt = np.zeros((HD, HD), dtype=np.float32)
    for d in range(HD // 2):
        rt[2 * d, 2 * d + 1] = 1.0
        rt[2 * d + 1, 2 * d] = -1.0
    rt = rt.astype(bf)

    ii = np.arange(128)[None, :]
    jj = np.arange(128)[:, None]
    tri = (ii >= jj).astype(np.float32)

    ones = np.ones((128, 1), dtype=np.float32)

    in_maps = []
    for core in range(8):
        b = core // 4
        g = core % 4
        qc = w_qkv[:, 512 * g: 512 * (g + 1)]
        kc = w_qkv[:, 2048 + 512 * g: 2048 + 512 * (g + 1)]
        vc = np.ascontiguousarray(w_qkv[:, 4096 + 512 * g: 4096 + 512 * (g + 1)]).astype(bf)
        wqk_c = np.concatenate([qc, kc], axis=1).astype(bf)
        wp_c = np.ascontiguousarray(w_proj[512 * g: 512 * (g + 1), :]).astype(bf)
        in_maps.append({
            "xT": xTb[b],
            "wqk": wqk_c,
            "wv": vc,
            "wp": wp_c,
            "cosP": cosP,
            "sinP": sinP,
            "rt": rt,
            "tri": tri,
            "ones": ones,
        })
    return in_maps


def _get_nc():
    if "nc" not in _CACHE:
        _CACHE["nc"] = _build_nc()
    return _CACHE["nc"]


def kernel(x, w_qkv, w_proj, freqs_cis, attn_mask, _trace=False):
    from concourse.bass_utils import run_bass_kernel_spmd

    in_maps = _host_prep(x, w_qkv, w_proj, freqs_cis)
    nc = _get_nc()
    res = run_bass_kernel_spmd(
        nc, in_maps, core_ids=list(range(8)), trace=_trace,
    )
    outs = [np.asarray(r["out"]).astype(np.float64) for r in res.results]
    full = np.stack([
        outs[0] + outs[1] + outs[2] + outs[3],
        outs[4] + outs[5] + outs[6] + outs[7],
    ]).astype(np.float32)
    if _trace:
        kernel._last_results = res
    return full


# revision 19
# speedup vs baseline: 1.2325x; 1.0249x over previous
"""Trainium2 Bass kernel for prefix-LM CausalSelfAttention.

Problem: B=2, T=2048, C=2048, H=16 heads (hd=128), prefix-LM mask
(bidirectional over first half, causal after), RoPE on q/k.

Sharding over 8 cores: data-parallel on batch (2) x tensor-parallel on
heads (4 heads per core). Each core computes a partial output projection
(its heads' contribution); partials (bf16) are summed on host.

Weight/activation matmul operands are bf16: bf16 stationary operands get
fast-weight-load (83ns vs 173ns f32r). Softmax probabilities stay f32r
(ACT-written bf16 streams ~40% slower as the MOVING matmul operand;
f32r moving with free dim >= 256 runs at 1 row/cycle). PSUM is f32.

Per-core dataflow:
  A. qT/kT = W_{q,k}^T @ x^T   [hd*4, T] transposed layout (head-major).
     DMAs are consolidated (one 3D DMA for all w k-tiles, one per x
     chunk) and split across the Sync and Activation DGE queues.
     PE warm-up matmuls run during the head DMA wait (p-state ramp).
  B. RoPE via pair-swap permutation matmul + DVE combine, interleaved
     with (C) so the PE stays fed while DVE does the rope math
  C. v = x @ Wv   [T, hd*4] natural layout (x tiles as stationary);
     v stays f32r (walrus rejects bf16-stationary x f32r-moving mixes)
  D. attention, query-chunk outer / head inner, software-pipelined:
     S(j+1) issues before PV(j) so the exp latency hides behind PE
     work. Per 512-wide chunk I:
       S'[J] = k_rope[:,J]^T-tile x q_rope[:,I]    (scores transposed)
       P'[J] = exp(S' / sqrt(hd))                  (ACT, PSUM->SBUF f32r)
       diagonal tiles: in-place DVE multiply of the single 128x128
       triangle block (mask is the same triangle for every diag tile)
       y_psum += v[J,h]^T-as-lhsT x P'[J]
       d_psum += ones^T x P'[J]                    (denominator)
     normalize y^T by broadcast fast-reciprocal of d
  E. partial out = yT^T-as-lhsT @ Wp; interleaved into the NEXT query
     chunk's attention stream; rows staged in SBUF as bf16 and DMA'd
     per 128-token row block (halves HBM write traffic).

Fully-masked key tiles are skipped (structural sparsity: 44/64 tiles/head).
"""
import math

import numpy as np

N_HEAD = 16
B = 2
T = 2048
C = 2048
HD = 128
HPC = 4          # heads per core
CL = HPC * HD    # local C = 512
TC = 512         # chunk width (matmul moving free dim / psum bank)
NT = T // TC     # 4 chunks
KT = C // 128    # 16 contraction tiles over C
TT = T // 128    # 16 T tiles
SCALE = 1.0 / math.sqrt(HD)

# Per query-chunk I: list of (J, mask_idx) key tiles to compute.
_JLISTS = {
    0: [(j, None) for j in range(8)],
    1: [(j, None) for j in range(8)],
    2: [(j, None) for j in range(8)] + [(8 + d, d) for d in range(4)],
    3: [(j, None) for j in range(12)] + [(12 + d, d) for d in range(4)],
}

_CACHE = {}


def _build_nc():
    import concourse.tile as tile
    import concourse.mybir as mybir
    from concourse import bacc

    f32 = mybir.dt.float32
    bf16 = mybir.dt.bfloat16

    nc = bacc.Bacc(None, target_bir_lowering=False)

    xT = nc.dram_tensor("xT", [C, T], bf16, kind="ExternalInput")
    wqk = nc.dram_tensor("wqk", [C, 2 * CL], bf16, kind="ExternalInput")
    wv = nc.dram_tensor("wv", [C, CL], bf16, kind="ExternalInput")
    wp = nc.dram_tensor("wp", [CL, C], bf16, kind="ExternalInput")
    cosP = nc.dram_tensor("cosP", [HD, T], bf16, kind="ExternalInput")
    sinP = nc.dram_tensor("sinP", [HD, T], bf16, kind="ExternalInput")
    f32r = mybir.dt.float32r
    rt = nc.dram_tensor("rt", [HD, HD], bf16, kind="ExternalInput")
    tri = nc.dram_tensor("tri", [128, 128], f32r, kind="ExternalInput")
    ones = nc.dram_tensor("ones", [128, 1], f32r, kind="ExternalInput")
    out = nc.dram_tensor("out", [T, C], bf16, kind="ExternalOutput")

    xT3 = xT.rearrange("(kt p) t -> p kt t", p=128)
    wqk3 = wqk.rearrange("(kt p) m -> p kt m", p=128)
    wv3 = wv.rearrange("(kt p) m -> p kt m", p=128)
    wp3 = wp.rearrange("(kt p) m -> p kt m", p=128)

    Exp = mybir.ActivationFunctionType.Exp

    with tile.TileContext(nc) as tc:
        mpool = tc.alloc_tile_pool(name="misc", bufs=1)
        qk_pool = tc.alloc_tile_pool(name="qkrope", bufs=1)
        tpool = tc.alloc_tile_pool(name="trig", bufs=1, side="right")
        xpool = tc.alloc_tile_pool(name="xt_sb", bufs=1, side="right")

        rt_sb = mpool.tile([HD, HD], bf16)
        ones_sb = mpool.tile([128, 1], f32r)
        tri_sb = mpool.tile([128, 128], f32r)
        cos_sb = tpool.tile([HD, T], bf16)
        sin_sb = tpool.tile([HD, T], bf16)
        warm_sb = mpool.tile([1, 1], f32)

        qkT = [qk_pool.tile([128, T], bf16, tag=f"qk{m}", name=f"qk{m}") for m in range(8)]

        # ---- stage A: qT/kT = W_{q,k}^T @ x^T, head-major tiles ----
        # Consolidated DMAs: rt first (tiny, gates PE warm-up), then the
        # critical x chunk-0 + all-w 3D DMAs, then x chunks 1..3. The
        # Activation DGE queue carries everything stage B/C/E needs.
        wpool = tc.alloc_tile_pool(name="wqk_sb", bufs=1)
        ps1 = tc.alloc_tile_pool(name="ps_qk", bufs=8, space="PSUM")
        # Many small interleaved DMAs on the sync queue, one tile per
        # k (baseline pattern — fine-grained DMA->matmul dependencies;
        # big shared tiles coarsen the tracking and stall stage A).
        w_t = []
        x_t = []
        for k in range(KT):
            wt = wpool.tile([128, 2 * CL], bf16, tag=f"w{k}", name=f"w{k}")
            nc.sync.dma_start(out=wt, in_=wqk3[:, k])
            w_t.append(wt)
            xt = xpool.tile([128, T], bf16, tag=f"x{k}", name=f"x{k}")
            nc.sync.dma_start(out=xt[:, 0:TC], in_=xT3[:, k, 0:TC])
            x_t.append(xt)
        for k in range(KT):
            nc.sync.dma_start(out=x_t[k][:, TC:], in_=xT3[:, k, TC:])
        # trig after the critical reads on the same queue: concurrent
        # queues contend for HBM and stall the first stage-A chains
        nc.sync.dma_start(out=rt_sb, in_=rt[:, :])
        nc.sync.dma_start(out=cos_sb, in_=cosP[:, :])
        nc.sync.dma_start(out=sin_sb, in_=sinP[:, :])

        # warm the ACT exp table during stage A (one-time ~2.7us load)
        nc.scalar.activation(out=warm_sb, in_=rt_sb[0:1, 0:1], func=Exp)

        for n in range(NT):
            nsl = slice(n * TC, (n + 1) * TC)
            for m in range(8):
                ps = ps1.tile([128, TC], f32, tag="ps_qk", name="ps_qk")
                for k in range(KT):
                    nc.tensor.matmul(
                        ps, w_t[k][:, m * 128:(m + 1) * 128], x_t[k][:, nsl],
                        start=(k == 0), stop=(k == KT - 1),
                    )
                nc.vector.tensor_copy(out=qkT[m][:, nsl], in_=ps)
        wpool.release()
        ps1.release()

        # ---- stage B+C interleaved: RoPE (PE tiny, DVE heavy) and
        # v = x @ Wv (PE heavy). Emitting v matmuls after each head's rope
        # keeps the PE busy while DVE works through the rope muls.
        v_pool = tc.alloc_tile_pool(name="v_sb", bufs=1)
        wvpool = tc.alloc_tile_pool(name="wv_sb", bufs=1)
        v_t = [v_pool.tile([128, CL], f32r, tag=f"v{mt}", name=f"v{mt}")
               for mt in range(TT)]
        # on sync: the scheduler hoists idle-engine DMA issues to t=0,
        # so putting these on the Activation queue makes their transfers
        # contend with the stage-A-critical w/x reads
        wv_t = []
        for k in range(KT):
            wt = wvpool.tile([128, CL], bf16, tag=f"wv{k}", name=f"wv{k}")
            nc.sync.dma_start(out=wt, in_=wv3[:, k])
            wv_t.append(wt)

        rope = [None] * 8
        rtmp = tc.alloc_tile_pool(name="rope_tmp", bufs=4)
        psr = tc.alloc_tile_pool(name="ps_rot", bufs=4, space="PSUM")
        ps2 = tc.alloc_tile_pool(name="ps_v", bufs=4, space="PSUM")

        def emit_v_pair(pair):
            for half in range(2):
                mt = 2 * pair + half
                tsl = slice(mt * 128, (mt + 1) * 128)
                ps = ps2.tile([128, CL], f32, tag="ps_v", name="ps_v")
                for k in range(KT):
                    nc.tensor.matmul(
                        ps, x_t[k][:, tsl], wv_t[k],
                        start=(k == 0), stop=(k == KT - 1),
                    )
                nc.scalar.copy(out=v_t[mt], in_=ps)

        for idx, m in enumerate((0, 4, 1, 5, 2, 6, 3, 7)):
            tmp = []
            for n in range(NT):
                sl = slice(n * TC, (n + 1) * TC)
                ps = psr.tile([128, TC], f32, tag="ps_rot", name="ps_rot")
                nc.tensor.matmul(ps, rt_sb, qkT[m][:, sl], start=True, stop=True)
                t1 = rtmp.tile([128, TC], bf16, tag="t1", name="t1")
                t2 = rtmp.tile([128, TC], bf16, tag="t2", name="t2")
                nc.vector.tensor_mul(t1, ps, sin_sb[:, sl])
                nc.vector.tensor_mul(t2, qkT[m][:, sl], cos_sb[:, sl])
                tmp.append((t1, t2))
            ro = qk_pool.tile([128, T], bf16, tag=f"qk{m}", name=f"rope{m}")
            for n in range(NT):
                sl = slice(n * TC, (n + 1) * TC)
                nc.vector.tensor_add(ro[:, sl], tmp[n][0], tmp[n][1])
            rope[m] = ro
            emit_v_pair(idx)
        ps2.release()
        psr.release()
        rtmp.release()
        wvpool.release()
        xpool.release()
        tpool.release()

        # ---- stage D attention (query-chunk outer) + stage E interleaved ----
        y_pool = tc.alloc_tile_pool(name="yT_sb", bufs=1)
        yT = [y_pool.tile([128, T], bf16, tag=f"yT{h}", name=f"yT{h}")
              for h in range(HPC)]
        wppool = tc.alloc_tile_pool(name="wp_sb", bufs=1)
        wp_t = []
        for hk in range(HPC):
            wt = wppool.tile([128, C], bf16, tag=f"wp{hk}", name=f"wp{hk}")
            nc.sync.dma_start(out=wt, in_=wp3[:, hk])
            wp_t.append(wt)
        nc.scalar.dma_start(out=ones_sb, in_=ones[:, :])
        nc.scalar.dma_start(out=tri_sb, in_=tri[:, :])

        pp_pool = tc.alloc_tile_pool(name="pp", bufs=6)
        sm_pool = tc.alloc_tile_pool(name="small", bufs=2)
        ps_s = tc.alloc_tile_pool(name="ps_s", bufs=3, space="PSUM")
        ps_y = tc.alloc_tile_pool(name="ps_y", bufs=2, space="PSUM")
        ps_o = tc.alloc_tile_pool(name="ps_o", bufs=1, space="PSUM")
        ps_d = tc.alloc_tile_pool(name="ps_d", bufs=2, space="PSUM")  # two halves
        opool = tc.alloc_tile_pool(name="ostage", bufs=2)

        ready_E = []     # mt values whose yT inputs are complete
        e_state = [None, 0]  # open [ot_tile, next_n] for current mt

        e_pools = [ps_o]
        e_ctr = [0]

        def emit_e_subgroup():
            # one (mt, n) block: 4 accumulating matmuls + copy to the
            # staged out row; DMA the full row after its 4th block.
            if e_state[0] is None:
                if not ready_E:
                    return
                e_state[0] = (ready_E.pop(0),
                              opool.tile([128, C], bf16, tag="ot", name="ot"))
                e_state[1] = 0
            mt, ot = e_state[0]
            n = e_state[1]
            msl = slice(mt * 128, (mt + 1) * 128)
            pool = e_pools[e_ctr[0] % len(e_pools)]
            e_ctr[0] += 1
            ps = pool.tile([128, TC], f32, tag="o", name="o_ps")
            for hk in range(HPC):
                nc.tensor.matmul(
                    ps, yT[hk][:, msl], wp_t[hk][:, n * TC:(n + 1) * TC],
                    start=(hk == 0), stop=(hk == HPC - 1),
                )
            nc.vector.tensor_copy(out=ot[:, n * TC:(n + 1) * TC], in_=ps)
            if mt >= TT - 4:
                # final blocks alternate across both DGE queues (both
                # near-idle by then) so the very last chunk lands on an
                # empty queue
                eng = nc.scalar if (e_ctr[0] % 2) else nc.sync
                eng.dma_start(out=out[msl, n * TC:(n + 1) * TC],
                              in_=ot[:, n * TC:(n + 1) * TC])
            e_state[1] += 1
            if e_state[1] == NT:
                if mt < TT - 4:
                    nc.sync.dma_start(out=out[msl, :], in_=ot)
                e_state[0] = None

        for I in range(NT):
            isl = slice(I * TC, (I + 1) * TC)
            jl = _JLISTS[I]
            for h in range(HPC):
                q_h = rope[h]
                k_h = rope[4 + h]
                y_ps = ps_y.tile([128, TC], f32, tag="y", name="y_ps")
                d_ps = ps_d.tile([1, TC], f32, tag="d", name="d_ps")
                pps = []

                def emit_S(jidx):
                    J, dmi = jl[jidx]
                    off = 0 if dmi is None else dmi * 128
                    osl = slice(off, TC)
                    s_ps = ps_s.tile([128, TC], f32, tag="s", name="s_ps")
                    nc.tensor.matmul(
                        s_ps[:, osl], k_h[:, J * 128:(J + 1) * 128],
                        q_h[:, I * TC + off:(I + 1) * TC],
                        start=True, stop=True,
                    )
                    pp = pp_pool.tile([128, TC], f32r, tag="pp", name="pp")
                    nc.scalar.activation(out=pp[:, osl], in_=s_ps[:, osl],
                                         func=Exp, scale=SCALE)
                    if dmi is not None:
                        # in-place masked multiply on the single 128x128
                        # triangle block at the streamed range's head
                        dsl = slice(off, off + 128)
                        nc.vector.tensor_mul(pp[:, dsl], pp[:, dsl], tri_sb)
                    pps.append((pp, osl, J, dmi))

                emit_S(0)
                for jidx in range(len(jl)):
                    if jidx + 1 < len(jl):
                        emit_S(jidx + 1)
                    pp, osl, J, dmi = pps[jidx]
                    if dmi is not None:
                        # extra PE fill while the DVE mask-mul lands
                        emit_e_subgroup()
                    first = jidx == 0
                    last = jidx == len(jl) - 1
                    nc.tensor.matmul(
                        y_ps[:, osl], v_t[J][:, h * 128:(h + 1) * 128],
                        pp[:, osl], start=first, stop=last,
                    )
                    nc.tensor.matmul(d_ps[:, osl], ones_sb, pp[:, osl],
                                     start=first, stop=last)
                    if jidx % 2 == 1 or len(ready_E) > 8:
                        emit_e_subgroup()
                recip = sm_pool.tile([1, TC], f32, tag="recip", name="recip")
                nc.vector.reciprocal_approx_fast(out=recip, in_=d_ps)
                recipB = sm_pool.tile([128, TC], f32, tag="recipB", name="recipB")
                nc.gpsimd.partition_broadcast(recipB, recip)
                nc.vector.tensor_mul(yT[h][:, isl], y_ps, recipB)
            # all heads' columns for chunk I are now complete
            ready_E.extend(range(4 * I, 4 * I + 4))
        ps_d.release()
        ps_o2 = tc.alloc_tile_pool(name="ps_o2", bufs=1, space="PSUM")
        e_pools.append(ps_o2)
        while ready_E or e_state[0] is not None:
            emit_e_subgroup()

        for p in (opool, sm_pool, pp_pool, wppool, y_pool, v_pool,
                  qk_pool, mpool, ps_o2, ps_o, ps_y, ps_s):
            p.release()
    nc.compile()
    return nc


def _host_prep(x, w_qkv, w_proj, freqs_cis):
    """Build per-core input maps (slicing + layout prep only)."""
    try:
        import ml_dtypes
        bf = ml_dtypes.bfloat16
    except ImportError:
        import jax.numpy as jnp
        bf = jnp.bfloat16
    x = np.asarray(x, dtype=np.float32)
    w_qkv = np.asarray(w_qkv, dtype=np.float32)
    w_proj = np.asarray(w_proj, dtype=np.float32)
    fc = np.asarray(freqs_cis, dtype=np.float32)

    xTb = [np.ascontiguousarray(x[b].T).astype(bf) for b in range(B)]

    cos = fc[:, :, 0].T  # [64, T]
    sin = fc[:, :, 1].T
    cosP = np.repeat(cos, 2, axis=0).astype(bf)  # [128, T]
    sinP = np.repeat(sin, 2, axis=0).astype(bf)

    rt = np.zeros((HD, HD), dtype=np.float32)
    for d in range(HD // 2):
        rt[2 * d, 2 * d + 1] = 1.0
        rt[2 * d + 1, 2 * d] = -1.0
    rt = rt.astype(bf)

    ii = np.arange(128)[None, :]
    jj = np.arange(128)[:, None]
    tri = (ii >= jj).astype(np.float32)

    ones = np.ones((128, 1), dtype=np.float32)

    in_maps = []
    for core in range(8):
        b = core // 4
        g = core % 4
        qc = w_qkv[:, 512 * g: 512 * (g + 1)]
        kc = w_qkv[:, 2048 + 512 * g: 2048 + 512 * (g + 1)]
        vc = np.ascontiguousarray(w_qkv[:, 4096 + 512 * g: 4096 + 512 * (g + 1)]).astype(bf)
        wqk_c = np.concatenate([qc, kc], axis=1).astype(bf)
        wp_c = np.ascontiguousarray(w_proj[512 * g: 512 * (g + 1), :]).astype(bf)
        in_maps.append({
            "xT": xTb[b],
            "wqk": wqk_c,
            "wv": vc,
            "wp": wp_c,
            "cosP": cosP,
            "sinP": sinP,
            "rt": rt,
            "tri": tri,
            "ones": ones,
        })
    return in_maps


def _get_nc():
    if "nc" not in _CACHE:
        _CACHE["nc"] = _build_nc()
    return _CACHE["nc"]


def kernel(x, w_qkv, w_proj, freqs_cis, attn_mask, _trace=False):
    from concourse.bass_utils import run_bass_kernel_spmd

    in_maps = _host_prep(x, w_qkv, w_proj, freqs_cis)
    nc = _get_nc()
    res = run_bass_kernel_spmd(
        nc, in_maps, core_ids=list(range(8)), trace=_trace,
    )
    outs = [np.asarray(r["out"]).astype(np.float64) for r in res.results]
    full = np.stack([
        outs[0] + outs[1] + outs[2] + outs[3],
        outs[4] + outs[5] + outs[6] + outs[7],
    ]).astype(np.float32)
    if _trace:
        kernel._last_results = res
    return full


# revision 41
# speedup vs baseline: 1.2335x; 1.0009x over previous
"""Trainium2 Bass kernel for prefix-LM CausalSelfAttention.

Problem: B=2, T=2048, C=2048, H=16 heads (hd=128), prefix-LM mask
(bidirectional over first half, causal after), RoPE on q/k.

Sharding over 8 cores: data-parallel on batch (2) x tensor-parallel on
heads (4 heads per core). Each core computes a partial output projection
(its heads' contribution); partials (bf16) are summed on host.

Weight/activation matmul operands are bf16: bf16 stationary operands get
fast-weight-load (83ns vs 173ns f32r). Softmax probabilities stay f32r
(ACT-written bf16 streams ~40% slower as the MOVING matmul operand;
f32r moving with free dim >= 256 runs at 1 row/cycle). PSUM is f32.

Per-core dataflow:
  A. qT/kT = W_{q,k}^T @ x^T   [hd*4, T] transposed layout (head-major).
     DMAs are consolidated (one 3D DMA for all w k-tiles, one per x
     chunk) and split across the Sync and Activation DGE queues.
     PE warm-up matmuls run during the head DMA wait (p-state ramp).
  B. RoPE via pair-swap permutation matmul + DVE combine, interleaved
     with (C) so the PE stays fed while DVE does the rope math
  C. v = x @ Wv   [T, hd*4] natural layout (x tiles as stationary);
     v stays f32r (walrus rejects bf16-stationary x f32r-moving mixes)
  D. attention, query-chunk outer / head inner, software-pipelined:
     S(j+1) issues before PV(j) so the exp latency hides behind PE
     work. Per 512-wide chunk I:
       S'[J] = k_rope[:,J]^T-tile x q_rope[:,I]    (scores transposed)
       P'[J] = exp(S' / sqrt(hd))                  (ACT, PSUM->SBUF f32r)
       diagonal tiles: in-place DVE multiply of the single 128x128
       triangle block (mask is the same triangle for every diag tile)
       y_psum += v[J,h]^T-as-lhsT x P'[J]
       d_psum += ones^T x P'[J]                    (denominator)
     normalize y^T by broadcast fast-reciprocal of d
  E. partial out = yT^T-as-lhsT @ Wp; interleaved into the NEXT query
     chunk's attention stream; rows staged in SBUF as bf16 and DMA'd
     per 128-token row block (halves HBM write traffic).

Fully-masked key tiles are skipped (structural sparsity: 44/64 tiles/head).
"""
import math

import numpy as np

N_HEAD = 16
B = 2
T = 2048
C = 2048
HD = 128
HPC = 4          # heads per core
CL = HPC * HD    # local C = 512
TC = 512         # chunk width (matmul moving free dim / psum bank)
NT = T // TC     # 4 chunks
KT = C // 128    # 16 contraction tiles over C
TT = T // 128    # 16 T tiles
SCALE = 1.0 / math.sqrt(HD)

# Per query-chunk I: list of (J, mask_idx) key tiles to compute.
_JLISTS = {
    0: [(j, None) for j in range(8)],
    1: [(j, None) for j in range(8)],
    2: [(j, None) for j in range(8)] + [(8 + d, d) for d in range(4)],
    3: [(j, None) for j in range(12)] + [(12 + d, d) for d in range(4)],
}

_CACHE = {}


def _build_nc():
    import concourse.tile as tile
    import concourse.mybir as mybir
    from concourse import bacc

    f32 = mybir.dt.float32
    bf16 = mybir.dt.bfloat16

    nc = bacc.Bacc(None, target_bir_lowering=False)

    xT = nc.dram_tensor("xT", [C, T], bf16, kind="ExternalInput")
    wqk = nc.dram_tensor("wqk", [C, 2 * CL], bf16, kind="ExternalInput")
    wv = nc.dram_tensor("wv", [C, CL], bf16, kind="ExternalInput")
    wp = nc.dram_tensor("wp", [CL, C], bf16, kind="ExternalInput")
    cosP = nc.dram_tensor("cosP", [HD, T], bf16, kind="ExternalInput")
    sinP = nc.dram_tensor("sinP", [HD, T], bf16, kind="ExternalInput")
    f32r = mybir.dt.float32r
    rt = nc.dram_tensor("rt", [HD, HD], bf16, kind="ExternalInput")
    tri = nc.dram_tensor("tri", [128, 128], f32r, kind="ExternalInput")
    ones = nc.dram_tensor("ones", [128, 1], f32r, kind="ExternalInput")
    out = nc.dram_tensor("out", [T, C], bf16, kind="ExternalOutput")

    xT3 = xT.rearrange("(kt p) t -> p kt t", p=128)
    wqk3 = wqk.rearrange("(kt p) m -> p kt m", p=128)
    wv3 = wv.rearrange("(kt p) m -> p kt m", p=128)
    wp3 = wp.rearrange("(kt p) m -> p kt m", p=128)

    Exp = mybir.ActivationFunctionType.Exp

    with tile.TileContext(nc) as tc:
        mpool = tc.alloc_tile_pool(name="misc", bufs=1)
        qk_pool = tc.alloc_tile_pool(name="qkrope", bufs=1)
        tpool = tc.alloc_tile_pool(name="trig", bufs=1, side="right")
        xpool = tc.alloc_tile_pool(name="xt_sb", bufs=1, side="right")

        rt_sb = mpool.tile([HD, HD], bf16)
        ones_sb = mpool.tile([128, 1], f32r)
        tri_sb = mpool.tile([128, 128], f32r)
        cos_sb = tpool.tile([HD, T], bf16)
        sin_sb = tpool.tile([HD, T], bf16)
        warm_sb = mpool.tile([1, 1], f32)

        qkT = [qk_pool.tile([128, T], bf16, tag=f"qk{m}", name=f"qk{m}") for m in range(8)]

        # ---- stage A: qT/kT = W_{q,k}^T @ x^T, head-major tiles ----
        # Consolidated DMAs: rt first (tiny, gates PE warm-up), then the
        # critical x chunk-0 + all-w 3D DMAs, then x chunks 1..3. The
        # Activation DGE queue carries everything stage B/C/E needs.
        wpool = tc.alloc_tile_pool(name="wqk_sb", bufs=1)
        ps1 = tc.alloc_tile_pool(name="ps_qk", bufs=8, space="PSUM")
        # Many small interleaved DMAs on the sync queue, one tile per
        # k (baseline pattern — fine-grained DMA->matmul dependencies;
        # big shared tiles coarsen the tracking and stall stage A).
        w_t = []
        x_t = []
        for k in range(KT):
            wt = wpool.tile([128, 2 * CL], bf16, tag=f"w{k}", name=f"w{k}")
            nc.sync.dma_start(out=wt, in_=wqk3[:, k])
            w_t.append(wt)
            xt = xpool.tile([128, T], bf16, tag=f"x{k}", name=f"x{k}")
            nc.sync.dma_start(out=xt[:, 0:TC], in_=xT3[:, k, 0:TC])
            x_t.append(xt)
        for k in range(KT):
            nc.sync.dma_start(out=x_t[k][:, TC:], in_=xT3[:, k, TC:])
        # trig after the critical reads on the same queue: concurrent
        # queues contend for HBM and stall the first stage-A chains
        nc.sync.dma_start(out=rt_sb, in_=rt[:, :])
        nc.sync.dma_start(out=cos_sb, in_=cosP[:, :])
        nc.sync.dma_start(out=sin_sb, in_=sinP[:, :])

        # warm the ACT exp table during stage A (one-time ~2.7us load)
        nc.scalar.activation(out=warm_sb, in_=rt_sb[0:1, 0:1], func=Exp)

        for n in range(NT):
            nsl = slice(n * TC, (n + 1) * TC)
            for m in range(8):
                ps = ps1.tile([128, TC], f32, tag="ps_qk", name="ps_qk")
                for k in range(KT):
                    nc.tensor.matmul(
                        ps, w_t[k][:, m * 128:(m + 1) * 128], x_t[k][:, nsl],
                        start=(k == 0), stop=(k == KT - 1),
                    )
                nc.vector.tensor_copy(out=qkT[m][:, nsl], in_=ps)
        wpool.release()
        ps1.release()

        # ---- stage B+C interleaved: RoPE (PE tiny, DVE heavy) and
        # v = x @ Wv (PE heavy). Emitting v matmuls after each head's rope
        # keeps the PE busy while DVE works through the rope muls.
        v_pool = tc.alloc_tile_pool(name="v_sb", bufs=1)
        wvpool = tc.alloc_tile_pool(name="wv_sb", bufs=1)
        v_t = [v_pool.tile([128, CL], f32r, tag=f"v{mt}", name=f"v{mt}")
               for mt in range(TT)]
        # on sync: the scheduler hoists idle-engine DMA issues to t=0,
        # so putting these on the Activation queue makes their transfers
        # contend with the stage-A-critical w/x reads
        wv_t = []
        for k in range(KT):
            wt = wvpool.tile([128, CL], bf16, tag=f"wv{k}", name=f"wv{k}")
            nc.sync.dma_start(out=wt, in_=wv3[:, k])
            wv_t.append(wt)

        rope = [None] * 8
        rtmp = tc.alloc_tile_pool(name="rope_tmp", bufs=4)
        psr = tc.alloc_tile_pool(name="ps_rot", bufs=4, space="PSUM")
        ps2 = tc.alloc_tile_pool(name="ps_v", bufs=4, space="PSUM")

        def emit_v_pair(pair):
            for half in range(2):
                mt = 2 * pair + half
                tsl = slice(mt * 128, (mt + 1) * 128)
                ps = ps2.tile([128, CL], f32, tag="ps_v", name="ps_v")
                for k in range(KT):
                    nc.tensor.matmul(
                        ps, x_t[k][:, tsl], wv_t[k],
                        start=(k == 0), stop=(k == KT - 1),
                    )
                nc.scalar.copy(out=v_t[mt], in_=ps)

        for idx, m in enumerate((0, 4, 1, 5, 2, 6, 3, 7)):
            tmp = []
            for n in range(NT):
                sl = slice(n * TC, (n + 1) * TC)
                ps = psr.tile([128, TC], f32, tag="ps_rot", name="ps_rot")
                nc.tensor.matmul(ps, rt_sb, qkT[m][:, sl], start=True, stop=True)
                t1 = rtmp.tile([128, TC], bf16, tag="t1", name="t1")
                t2 = rtmp.tile([128, TC], bf16, tag="t2", name="t2")
                nc.vector.tensor_mul(t1, ps, sin_sb[:, sl])
                nc.vector.tensor_mul(t2, qkT[m][:, sl], cos_sb[:, sl])
                tmp.append((t1, t2))
            ro = qk_pool.tile([128, T], bf16, tag=f"qk{m}", name=f"rope{m}")
            for n in range(NT):
                sl = slice(n * TC, (n + 1) * TC)
                nc.vector.tensor_add(ro[:, sl], tmp[n][0], tmp[n][1])
            rope[m] = ro
            emit_v_pair(idx)
        ps2.release()
        psr.release()
        rtmp.release()
        wvpool.release()
        xpool.release()
        tpool.release()

        # ---- stage D attention (query-chunk outer) + stage E interleaved ----
        y_pool = tc.alloc_tile_pool(name="yT_sb", bufs=1)
        yT = [y_pool.tile([128, T], bf16, tag=f"yT{h}", name=f"yT{h}")
              for h in range(HPC)]
        wppool = tc.alloc_tile_pool(name="wp_sb", bufs=1)
        wp_t = []
        for hk in range(HPC):
            wt = wppool.tile([128, C], bf16, tag=f"wp{hk}", name=f"wp{hk}")
            nc.sync.dma_start(out=wt, in_=wp3[:, hk])
            wp_t.append(wt)
        nc.scalar.dma_start(out=ones_sb, in_=ones[:, :])
        nc.scalar.dma_start(out=tri_sb, in_=tri[:, :])

        pp_pool = tc.alloc_tile_pool(name="pp", bufs=3)
        sm_pool = tc.alloc_tile_pool(name="small", bufs=2)
        # S psums are [128, 2*TC] (2 banks): two key-tiles share ONE exp
        # ACT op, halving ACT instruction+sem overhead
        ps_s = tc.alloc_tile_pool(name="ps_s", bufs=2, space="PSUM")
        ps_y = tc.alloc_tile_pool(name="ps_y", bufs=2, space="PSUM")
        ps_o = tc.alloc_tile_pool(name="ps_o", bufs=1, space="PSUM")
        # one denominator bank, cycled per head: the reciprocal read
        # completes before the next head's first d-matmul issues (the
        # pair pipeline leaves >= 4 PE matmuls in between)
        ps_d = tc.alloc_tile_pool(name="ps_d", bufs=1, space="PSUM")
        opool = tc.alloc_tile_pool(name="ostage", bufs=2)

        ready_E = []     # mt values whose yT inputs are complete
        e_state = [None, 0]  # open [ot_tile, next_n] for current mt

        e_pools = [ps_o]
        e_ctr = [0]

        def emit_e_subgroup():
            # one (mt, n) block: 4 accumulating matmuls + copy to the
            # staged out row; DMA the full row after its 4th block.
            if e_state[0] is None:
                if not ready_E:
                    return
                e_state[0] = (ready_E.pop(0),
                              opool.tile([128, C], bf16, tag="ot", name="ot"))
                e_state[1] = 0
            mt, ot = e_state[0]
            n = e_state[1]
            msl = slice(mt * 128, (mt + 1) * 128)
            pool = e_pools[e_ctr[0] % len(e_pools)]
            e_ctr[0] += 1
            ps = pool.tile([128, TC], f32, tag="o", name="o_ps")
            for hk in range(HPC):
                nc.tensor.matmul(
                    ps, yT[hk][:, msl], wp_t[hk][:, n * TC:(n + 1) * TC],
                    start=(hk == 0), stop=(hk == HPC - 1),
                )
            nc.vector.tensor_copy(out=ot[:, n * TC:(n + 1) * TC], in_=ps)
            if mt >= TT - 4:
                # final blocks alternate across both DGE queues (both
                # near-idle by then) so the very last chunk lands on an
                # empty queue
                eng = nc.scalar if (e_ctr[0] % 2) else nc.sync
                eng.dma_start(out=out[msl, n * TC:(n + 1) * TC],
                              in_=ot[:, n * TC:(n + 1) * TC])
            e_state[1] += 1
            if e_state[1] == NT:
                if mt < TT - 4:
                    nc.sync.dma_start(out=out[msl, :], in_=ot)
                e_state[0] = None

        for I in range(NT):
            isl = slice(I * TC, (I + 1) * TC)
            jl = _JLISTS[I]
            for h in range(HPC):
                q_h = rope[h]
                k_h = rope[4 + h]
                y_ps = ps_y.tile([128, TC], f32, tag="y", name="y_ps")
                d_ps = ps_d.tile([1, TC], f32, tag="d", name="d_ps")
                npairs = len(jl) // 2
                pps = []

                def emit_S_pair(p):
                    s2 = ps_s.tile([128, 2 * TC], f32, tag="s", name="s_ps")
                    pp2 = pp_pool.tile([128, 2 * TC], f32r, tag="pp", name="pp")
                    los = []
                    for half in range(2):
                        J, dmi = jl[2 * p + half]
                        off = 0 if dmi is None else dmi * 128
                        base = half * TC
                        nc.tensor.matmul(
                            s2[:, base + off:base + TC],
                            k_h[:, J * 128:(J + 1) * 128],
                            q_h[:, I * TC + off:(I + 1) * TC],
                            start=True, stop=True,
                        )
                        los.append((J, dmi, off, base))
                    # one exp over both banks; trimmed head columns read
                    # stale psum -> garbage pp values that are never read
                    nc.scalar.activation(out=pp2, in_=s2, func=Exp,
                                         scale=SCALE)
                    for (J, dmi, off, base) in los:
                        if dmi is not None:
                            # in-place masked multiply on the single
                            # 128x128 triangle block
                            dsl = slice(base + off, base + off + 128)
                            nc.vector.tensor_mul(pp2[:, dsl], pp2[:, dsl],
                                                 tri_sb)
                    pps.append((pp2, los))

                emit_S_pair(0)
                for p in range(npairs):
                    if p + 1 < npairs:
                        emit_S_pair(p + 1)
                    pp2, los = pps[p]
                    for hi, (J, dmi, off, base) in enumerate(los):
                        if dmi is not None:
                            # extra PE fill while the DVE mask-mul lands
                            emit_e_subgroup()
                        first = p == 0 and hi == 0
                        last = p == npairs - 1 and hi == 1
                        nc.tensor.matmul(
                            y_ps[:, off:TC],
                            v_t[J][:, h * 128:(h + 1) * 128],
                            pp2[:, base + off:base + TC],
                            start=first, stop=last,
                        )
                        nc.tensor.matmul(d_ps[:, off:TC], ones_sb,
                                         pp2[:, base + off:base + TC],
                                         start=first, stop=last)
                    if len(ready_E) > 0:
                        emit_e_subgroup()
                recip = sm_pool.tile([1, TC], f32, tag="recip", name="recip")
                nc.vector.reciprocal_approx_fast(out=recip, in_=d_ps)
                recipB = sm_pool.tile([128, TC], f32, tag="recipB", name="recipB")
                nc.gpsimd.partition_broadcast(recipB, recip)
                nc.vector.tensor_mul(yT[h][:, isl], y_ps, recipB)
            # all heads' columns for chunk I are now complete
            ready_E.extend(range(4 * I, 4 * I + 4))
        ps_d.release()
        ps_o2 = tc.alloc_tile_pool(name="ps_o2", bufs=1, space="PSUM")
        e_pools.append(ps_o2)
        while ready_E or e_state[0] is not None:
            emit_e_subgroup()

        for p in (opool, sm_pool, pp_pool, wppool, y_pool, v_pool,
                  qk_pool, mpool, ps_o2, ps_o, ps_y, ps_s):
            p.release()
    nc.compile()
    return nc


def _host_prep(x, w_qkv, w_proj, freqs_cis):
    """Build per-core input maps (slicing + layout prep only)."""
    try:
        import ml_dtypes
        bf = ml_dtypes.bfloat16
    except ImportError:
        import jax.numpy as jnp
        bf = jnp.bfloat16
    x = np.asarray(x, dtype=np.float32)
    w_qkv = np.asarray(w_qkv, dtype=np.float32)
    w_proj = np.asarray(w_proj, dtype=np.float32)
    fc = np.asarray(freqs_cis, dtype=np.float32)

    xTb = [np.ascontiguousarray(x[b].T).astype(bf) for b in range(B)]

    cos = fc[:, :, 0].T  # [64, T]
    sin = fc[:, :, 1].T
    cosP = np.repeat(cos, 2, axis=0).astype(bf)  # [128, T]
    sinP = np.repeat(sin, 2, axis=0).astype(bf)

    rt = np.zeros((HD, HD), dtype=np.float32)
    for d in range(HD // 2):
        rt[2 * d, 2 * d + 1] = 1.0
        rt[2 * d + 1, 2 * d] = -1.0
    rt = rt.astype(bf)

    ii = np.arange(128)[None, :]
    jj = np.arange(128)[:, None]
    tri = (ii >= jj).astype(np.float32)

    ones = np.ones((128, 1), dtype=np.float32)

    in_maps = []
    for core in range(8):
        b = core // 4
        g = core % 4
        qc = w_qkv[:, 512 * g: 512 * (g + 1)]
        kc = w_qkv[:, 2048 + 512 * g: 2048 + 512 * (g + 1)]
        vc = np.ascontiguousarray(w_qkv[:, 4096 + 512 * g: 4096 + 512 * (g + 1)]).astype(bf)
        wqk_c = np.concatenate([qc, kc], axis=1).astype(bf)
        wp_c = np.ascontiguousarray(w_proj[512 * g: 512 * (g + 1), :]).astype(bf)
        in_maps.append({
            "xT": xTb[b],
            "wqk": wqk_c,
            "wv": vc,
            "wp": wp_c,
            "cosP": cosP,
            "sinP": sinP,
            "rt": rt,
            "tri": tri,
            "ones": ones,
        })
    return in_maps


def _get_nc():
    if "nc" not in _CACHE:
        _CACHE["nc"] = _build_nc()
    return _CACHE["nc"]


def kernel(x, w_qkv, w_proj, freqs_cis, attn_mask, _trace=False):
    from concourse.bass_utils import run_bass_kernel_spmd

    in_maps = _host_prep(x, w_qkv, w_proj, freqs_cis)
    nc = _get_nc()
    res = run_bass_kernel_spmd(
        nc, in_maps, core_ids=list(range(8)), trace=_trace,
    )
    outs = [np.asarray(r["out"]).astype(np.float64) for r in res.results]
    full = np.stack([
        outs[0] + outs[1] + outs[2] + outs[3],
        outs[4] + outs[5] + outs[6] + outs[7],
    ]).astype(np.float32)
    if _trace:
        kernel._last_results = res
    return full


# revision 43
# speedup vs baseline: 1.2365x; 1.0024x over previous
"""Trainium2 Bass kernel for prefix-LM CausalSelfAttention.

Problem: B=2, T=2048, C=2048, H=16 heads (hd=128), prefix-LM mask
(bidirectional over first half, causal after), RoPE on q/k.

Sharding over 8 cores: data-parallel on batch (2) x tensor-parallel on
heads (4 heads per core). Each core computes a partial output projection
(its heads' contribution); partials (bf16) are summed on host.

Weight/activation matmul operands are bf16: bf16 stationary operands get
fast-weight-load (83ns vs 173ns f32r). Softmax probabilities stay f32r
(ACT-written bf16 streams ~40% slower as the MOVING matmul operand;
f32r moving with free dim >= 256 runs at 1 row/cycle). PSUM is f32.

Per-core dataflow:
  A. qT/kT = W_{q,k}^T @ x^T   [hd*4, T] transposed layout (head-major).
     DMAs are consolidated (one 3D DMA for all w k-tiles, one per x
     chunk) and split across the Sync and Activation DGE queues.
     PE warm-up matmuls run during the head DMA wait (p-state ramp).
  B. RoPE via pair-swap permutation matmul + DVE combine, interleaved
     with (C) so the PE stays fed while DVE does the rope math
  C. v = x @ Wv   [T, hd*4] natural layout (x tiles as stationary);
     v stays f32r (walrus rejects bf16-stationary x f32r-moving mixes)
  D. attention, query-chunk outer / head inner, software-pipelined:
     S(j+1) issues before PV(j) so the exp latency hides behind PE
     work. Per 512-wide chunk I:
       S'[J] = k_rope[:,J]^T-tile x q_rope[:,I]    (scores transposed)
       P'[J] = exp(S' / sqrt(hd))                  (ACT, PSUM->SBUF f32r)
       diagonal tiles: in-place DVE multiply of the single 128x128
       triangle block (mask is the same triangle for every diag tile)
       y_psum += v[J,h]^T-as-lhsT x P'[J]
       d_psum += ones^T x P'[J]                    (denominator)
     normalize y^T by broadcast fast-reciprocal of d
  E. partial out = yT^T-as-lhsT @ Wp; interleaved into the NEXT query
     chunk's attention stream; rows staged in SBUF as bf16 and DMA'd
     per 128-token row block (halves HBM write traffic).

Fully-masked key tiles are skipped (structural sparsity: 44/64 tiles/head).
"""
import math

import numpy as np

N_HEAD = 16
B = 2
T = 2048
C = 2048
HD = 128
HPC = 4          # heads per core
CL = HPC * HD    # local C = 512
TC = 512         # chunk width (matmul moving free dim / psum bank)
NT = T // TC     # 4 chunks
KT = C // 128    # 16 contraction tiles over C
TT = T // 128    # 16 T tiles
SCALE = 1.0 / math.sqrt(HD)

# Per query-chunk I: list of (J, mask_idx) key tiles to compute.
_JLISTS = {
    0: [(j, None) for j in range(8)],
    1: [(j, None) for j in range(8)],
    2: [(j, None) for j in range(8)] + [(8 + d, d) for d in range(4)],
    3: [(j, None) for j in range(12)] + [(12 + d, d) for d in range(4)],
}

_CACHE = {}


def _build_nc():
    import concourse.tile as tile
    import concourse.mybir as mybir
    from concourse import bacc

    f32 = mybir.dt.float32
    bf16 = mybir.dt.bfloat16

    nc = bacc.Bacc(None, target_bir_lowering=False)

    xT = nc.dram_tensor("xT", [C, T], bf16, kind="ExternalInput")
    wqk = nc.dram_tensor("wqk", [C, 2 * CL], bf16, kind="ExternalInput")
    wv = nc.dram_tensor("wv", [C, CL], bf16, kind="ExternalInput")
    wp = nc.dram_tensor("wp", [CL, C], bf16, kind="ExternalInput")
    cosP = nc.dram_tensor("cosP", [HD, T], bf16, kind="ExternalInput")
    sinP = nc.dram_tensor("sinP", [HD, T], bf16, kind="ExternalInput")
    f32r = mybir.dt.float32r
    rt = nc.dram_tensor("rt", [HD, HD], bf16, kind="ExternalInput")
    tri = nc.dram_tensor("tri", [128, 128], f32r, kind="ExternalInput")
    ones = nc.dram_tensor("ones", [128, 1], f32r, kind="ExternalInput")
    out = nc.dram_tensor("out", [T, C], bf16, kind="ExternalOutput")

    xT3 = xT.rearrange("(kt p) t -> p kt t", p=128)
    wqk3 = wqk.rearrange("(kt p) m -> p kt m", p=128)
    wv3 = wv.rearrange("(kt p) m -> p kt m", p=128)
    wp3 = wp.rearrange("(kt p) m -> p kt m", p=128)

    Exp = mybir.ActivationFunctionType.Exp

    with tile.TileContext(nc) as tc:
        mpool = tc.alloc_tile_pool(name="misc", bufs=1)
        qk_pool = tc.alloc_tile_pool(name="qkrope", bufs=1)
        tpool = tc.alloc_tile_pool(name="trig", bufs=1, side="right")
        xpool = tc.alloc_tile_pool(name="xt_sb", bufs=1, side="right")

        rt_sb = mpool.tile([HD, HD], bf16)
        ones_sb = mpool.tile([128, 1], f32r)
        tri_sb = mpool.tile([128, 128], f32r)
        cos_sb = tpool.tile([HD, T], bf16)
        sin_sb = tpool.tile([HD, T], bf16)
        warm_sb = mpool.tile([1, 1], f32)

        qkT = [qk_pool.tile([128, T], bf16, tag=f"qk{m}", name=f"qk{m}") for m in range(8)]

        # ---- stage A: qT/kT = W_{q,k}^T @ x^T, head-major tiles ----
        # Consolidated DMAs: rt first (tiny, gates PE warm-up), then the
        # critical x chunk-0 + all-w 3D DMAs, then x chunks 1..3. The
        # Activation DGE queue carries everything stage B/C/E needs.
        wpool = tc.alloc_tile_pool(name="wqk_sb", bufs=1)
        ps1 = tc.alloc_tile_pool(name="ps_qk", bufs=8, space="PSUM")
        # Many small interleaved DMAs on the sync queue, one tile per
        # k (baseline pattern — fine-grained DMA->matmul dependencies;
        # big shared tiles coarsen the tracking and stall stage A).
        w_t = []
        x_t = []
        for k in range(KT):
            wt = wpool.tile([128, 2 * CL], bf16, tag=f"w{k}", name=f"w{k}")
            nc.sync.dma_start(out=wt, in_=wqk3[:, k])
            w_t.append(wt)
            xt = xpool.tile([128, T], bf16, tag=f"x{k}", name=f"x{k}")
            nc.sync.dma_start(out=xt[:, 0:TC], in_=xT3[:, k, 0:TC])
            x_t.append(xt)
        for k in range(KT):
            nc.sync.dma_start(out=x_t[k][:, TC:], in_=xT3[:, k, TC:])
        # trig after the critical reads on the same queue: concurrent
        # queues contend for HBM and stall the first stage-A chains
        nc.sync.dma_start(out=rt_sb, in_=rt[:, :])
        nc.sync.dma_start(out=cos_sb, in_=cosP[:, :])
        nc.sync.dma_start(out=sin_sb, in_=sinP[:, :])

        # warm the ACT exp table during stage A (one-time ~2.7us load)
        nc.scalar.activation(out=warm_sb, in_=rt_sb[0:1, 0:1], func=Exp)

        for n in range(NT):
            nsl = slice(n * TC, (n + 1) * TC)
            for m in range(8):
                ps = ps1.tile([128, TC], f32, tag="ps_qk", name="ps_qk")
                for k in range(KT):
                    nc.tensor.matmul(
                        ps, w_t[k][:, m * 128:(m + 1) * 128], x_t[k][:, nsl],
                        start=(k == 0), stop=(k == KT - 1),
                    )
                nc.vector.tensor_copy(out=qkT[m][:, nsl], in_=ps)
        wpool.release()
        ps1.release()

        # ---- stage B+C interleaved: RoPE (PE tiny, DVE heavy) and
        # v = x @ Wv (PE heavy). Emitting v matmuls after each head's rope
        # keeps the PE busy while DVE works through the rope muls.
        v_pool = tc.alloc_tile_pool(name="v_sb", bufs=1)
        wvpool = tc.alloc_tile_pool(name="wv_sb", bufs=1)
        v_t = [v_pool.tile([128, CL], f32r, tag=f"v{mt}", name=f"v{mt}")
               for mt in range(TT)]
        # on sync: the scheduler hoists idle-engine DMA issues to t=0,
        # so putting these on the Activation queue makes their transfers
        # contend with the stage-A-critical w/x reads
        wv_t = []
        for k in range(KT):
            wt = wvpool.tile([128, CL], bf16, tag=f"wv{k}", name=f"wv{k}")
            nc.sync.dma_start(out=wt, in_=wv3[:, k])
            wv_t.append(wt)

        rope = [None] * 8
        rtmp = tc.alloc_tile_pool(name="rope_tmp", bufs=4)
        psr = tc.alloc_tile_pool(name="ps_rot", bufs=4, space="PSUM")
        ps2 = tc.alloc_tile_pool(name="ps_v", bufs=4, space="PSUM")

        def emit_v_pair(pair):
            for half in range(2):
                mt = 2 * pair + half
                tsl = slice(mt * 128, (mt + 1) * 128)
                ps = ps2.tile([128, CL], f32, tag="ps_v", name="ps_v")
                for k in range(KT):
                    nc.tensor.matmul(
                        ps, x_t[k][:, tsl], wv_t[k],
                        start=(k == 0), stop=(k == KT - 1),
                    )
                nc.scalar.copy(out=v_t[mt], in_=ps)

        for idx, m in enumerate((0, 4, 1, 5, 2, 6, 3, 7)):
            tmp = []
            for n in range(NT):
                sl = slice(n * TC, (n + 1) * TC)
                ps = psr.tile([128, TC], f32, tag="ps_rot", name="ps_rot")
                nc.tensor.matmul(ps, rt_sb, qkT[m][:, sl], start=True, stop=True)
                t1 = rtmp.tile([128, TC], bf16, tag="t1", name="t1")
                t2 = rtmp.tile([128, TC], bf16, tag="t2", name="t2")
                nc.vector.tensor_mul(t1, ps, sin_sb[:, sl])
                nc.vector.tensor_mul(t2, qkT[m][:, sl], cos_sb[:, sl])
                tmp.append((t1, t2))
            ro = qk_pool.tile([128, T], bf16, tag=f"qk{m}", name=f"rope{m}")
            for n in range(NT):
                sl = slice(n * TC, (n + 1) * TC)
                nc.vector.tensor_add(ro[:, sl], tmp[n][0], tmp[n][1])
            rope[m] = ro
            emit_v_pair(idx)
        ps2.release()
        psr.release()
        rtmp.release()
        wvpool.release()
        xpool.release()
        tpool.release()

        # ---- stage D attention (query-chunk outer) + stage E interleaved ----
        y_pool = tc.alloc_tile_pool(name="yT_sb", bufs=1)
        yT = [y_pool.tile([128, T], bf16, tag=f"yT{h}", name=f"yT{h}")
              for h in range(HPC)]
        wppool = tc.alloc_tile_pool(name="wp_sb", bufs=1)
        wp_t = []
        for hk in range(HPC):
            wt = wppool.tile([128, C], bf16, tag=f"wp{hk}", name=f"wp{hk}")
            nc.sync.dma_start(out=wt, in_=wp3[:, hk])
            wp_t.append(wt)
        nc.scalar.dma_start(out=ones_sb, in_=ones[:, :])
        nc.scalar.dma_start(out=tri_sb, in_=tri[:, :])

        pp_pool = tc.alloc_tile_pool(name="pp", bufs=4)
        sm_pool = tc.alloc_tile_pool(name="small", bufs=2)
        # S psums are [128, 2*TC] (2 banks): two key-tiles share ONE exp
        # ACT op, halving ACT instruction+sem overhead
        ps_s = tc.alloc_tile_pool(name="ps_s", bufs=2, space="PSUM")
        ps_y = tc.alloc_tile_pool(name="ps_y", bufs=2, space="PSUM")
        ps_o = tc.alloc_tile_pool(name="ps_o", bufs=1, space="PSUM")
        # one denominator bank, cycled per head: the reciprocal read
        # completes before the next head's first d-matmul issues (the
        # pair pipeline leaves >= 4 PE matmuls in between)
        ps_d = tc.alloc_tile_pool(name="ps_d", bufs=1, space="PSUM")
        opool = tc.alloc_tile_pool(name="ostage", bufs=2)

        ready_E = []     # mt values whose yT inputs are complete
        e_state = [None, 0]  # open [ot_tile, next_n] for current mt

        e_pools = [ps_o]
        e_ctr = [0]

        def emit_e_subgroup():
            # one (mt, n) block: 4 accumulating matmuls + copy to the
            # staged out row; DMA the full row after its 4th block.
            if e_state[0] is None:
                if not ready_E:
                    return
                e_state[0] = (ready_E.pop(0),
                              opool.tile([128, C], bf16, tag="ot", name="ot"))
                e_state[1] = 0
            mt, ot = e_state[0]
            n = e_state[1]
            msl = slice(mt * 128, (mt + 1) * 128)
            pool = e_pools[e_ctr[0] % len(e_pools)]
            e_ctr[0] += 1
            ps = pool.tile([128, TC], f32, tag="o", name="o_ps")
            for hk in range(HPC):
                nc.tensor.matmul(
                    ps, yT[hk][:, msl], wp_t[hk][:, n * TC:(n + 1) * TC],
                    start=(hk == 0), stop=(hk == HPC - 1),
                )
            nc.vector.tensor_copy(out=ot[:, n * TC:(n + 1) * TC], in_=ps)
            if mt >= TT - 4:
                # final blocks alternate across both DGE queues (both
                # near-idle by then) so the very last chunk lands on an
                # empty queue
                eng = nc.scalar if (e_ctr[0] % 2) else nc.sync
                eng.dma_start(out=out[msl, n * TC:(n + 1) * TC],
                              in_=ot[:, n * TC:(n + 1) * TC])
            e_state[1] += 1
            if e_state[1] == NT:
                if mt < TT - 4:
                    nc.sync.dma_start(out=out[msl, :], in_=ot)
                e_state[0] = None

        for I in range(NT):
            isl = slice(I * TC, (I + 1) * TC)
            jl = _JLISTS[I]
            for h in range(HPC):
                q_h = rope[h]
                k_h = rope[4 + h]
                y_ps = ps_y.tile([128, TC], f32, tag="y", name="y_ps")
                d_ps = ps_d.tile([1, TC], f32, tag="d", name="d_ps")
                npairs = len(jl) // 2
                pps = []

                def emit_S_pair(p):
                    s2 = ps_s.tile([128, 2 * TC], f32, tag="s", name="s_ps")
                    pp2 = pp_pool.tile([128, 2 * TC], f32r, tag="pp", name="pp")
                    los = []
                    for half in range(2):
                        J, dmi = jl[2 * p + half]
                        off = 0 if dmi is None else dmi * 128
                        base = half * TC
                        nc.tensor.matmul(
                            s2[:, base + off:base + TC],
                            k_h[:, J * 128:(J + 1) * 128],
                            q_h[:, I * TC + off:(I + 1) * TC],
                            start=True, stop=True,
                        )
                        los.append((J, dmi, off, base))
                    # one exp over both banks; trimmed head columns read
                    # stale psum -> garbage pp values that are never read
                    nc.scalar.activation(out=pp2, in_=s2, func=Exp,
                                         scale=SCALE)
                    for (J, dmi, off, base) in los:
                        if dmi is not None:
                            # in-place masked multiply on the single
                            # 128x128 triangle block
                            dsl = slice(base + off, base + off + 128)
                            nc.vector.tensor_mul(pp2[:, dsl], pp2[:, dsl],
                                                 tri_sb)
                    pps.append((pp2, los))

                emit_S_pair(0)
                for p in range(npairs):
                    if p + 1 < npairs:
                        emit_S_pair(p + 1)
                    pp2, los = pps[p]
                    for hi, (J, dmi, off, base) in enumerate(los):
                        if dmi is not None:
                            # extra PE fill while the DVE mask-mul lands
                            emit_e_subgroup()
                        first = p == 0 and hi == 0
                        last = p == npairs - 1 and hi == 1
                        nc.tensor.matmul(
                            y_ps[:, off:TC],
                            v_t[J][:, h * 128:(h + 1) * 128],
                            pp2[:, base + off:base + TC],
                            start=first, stop=last,
                        )
                        nc.tensor.matmul(d_ps[:, off:TC], ones_sb,
                                         pp2[:, base + off:base + TC],
                                         start=first, stop=last)
                    # concentrate E filler late in each chunk: the early
                    # pairs have S-work to hide latency, the diag/late
                    # pairs are the stall-prone ones
                    if p >= npairs - 2:
                        emit_e_subgroup()
                recip = sm_pool.tile([1, TC], f32, tag="recip", name="recip")
                nc.vector.reciprocal_approx_fast(out=recip, in_=d_ps)
                recipB = sm_pool.tile([128, TC], f32, tag="recipB", name="recipB")
                nc.gpsimd.partition_broadcast(recipB, recip)
                nc.vector.tensor_mul(yT[h][:, isl], y_ps, recipB)
            # all heads' columns for chunk I are now complete
            ready_E.extend(range(4 * I, 4 * I + 4))
        ps_d.release()
        ps_o2 = tc.alloc_tile_pool(name="ps_o2", bufs=1, space="PSUM")
        e_pools.append(ps_o2)
        while ready_E or e_state[0] is not None:
            emit_e_subgroup()

        for p in (opool, sm_pool, pp_pool, wppool, y_pool, v_pool,
                  qk_pool, mpool, ps_o2, ps_o, ps_y, ps_s):
            p.release()
    nc.compile()
    return nc


def _host_prep(x, w_qkv, w_proj, freqs_cis):
    """Build per-core input maps (slicing + layout prep only)."""
    try:
        import ml_dtypes
        bf = ml_dtypes.bfloat16
    except ImportError:
        import jax.numpy as jnp
        bf = jnp.bfloat16
    x = np.asarray(x, dtype=np.float32)
    w_qkv = np.asarray(w_qkv, dtype=np.float32)
    w_proj = np.asarray(w_proj, dtype=np.float32)
    fc = np.asarray(freqs_cis, dtype=np.float32)

    xTb = [np.ascontiguousarray(x[b].T).astype(bf) for b in range(B)]

    cos = fc[:, :, 0].T  # [64, T]
    sin = fc[:, :, 1].T
    cosP = np.repeat(cos, 2, axis=0).astype(bf)  # [128, T]
    sinP = np.repeat(sin, 2, axis=0).astype(bf)

    rt = np.zeros((HD, HD), dtype=np.float32)
    for d in range(HD // 2):
        rt[2 * d, 2 * d + 1] = 1.0
        rt[2 * d + 1, 2 * d] = -1.0
    rt = rt.astype(bf)

    ii = np.arange(128)[None, :]
    jj = np.arange(128)[:, None]
    tri = (ii >= jj).astype(np.float32)

    ones = np.ones((128, 1), dtype=np.float32)

    in_maps = []
    for core in range(8):
        b = core // 4
        g = core % 4
        qc = w_qkv[:, 512 * g: 512 * (g + 1)]
        kc = w_qkv[:, 2048 + 512 * g: 2048 + 512 * (g + 1)]
        vc = np.ascontiguousarray(w_qkv[:, 4096 + 512 * g: 4096 + 512 * (g + 1)]).astype(bf)
        wqk_c = np.concatenate([qc, kc], axis=1).astype(bf)
        wp_c = np.ascontiguousarray(w_proj[512 * g: 512 * (g + 1), :]).astype(bf)
        in_maps.append({
            "xT": xTb[b],
            "wqk": wqk_c,
            "wv": vc,
            "wp": wp_c,
            "cosP": cosP,
            "sinP": sinP,
            "rt": rt,
            "tri": tri,
            "ones": ones,
        })
    return in_maps


def _get_nc():
    if "nc" not in _CACHE:
        _CACHE["nc"] = _build_nc()
    return _CACHE["nc"]


def kernel(x, w_qkv, w_proj, freqs_cis, attn_mask, _trace=False):
    from concourse.bass_utils import run_bass_kernel_spmd

    in_maps = _host_prep(x, w_qkv, w_proj, freqs_cis)
    nc = _get_nc()
    res = run_bass_kernel_spmd(
        nc, in_maps, core_ids=list(range(8)), trace=_trace,
    )
    outs = [np.asarray(r["out"]).astype(np.float64) for r in res.results]
    full = np.stack([
        outs[0] + outs[1] + outs[2] + outs[3],
        outs[4] + outs[5] + outs[6] + outs[7],
    ]).astype(np.float32)
    if _trace:
        kernel._last_results = res
    return full
